# revision 1
# baseline (speedup 1.0000x reference)
"""Trainium2 Bass kernel for a 6-layer post-BatchNorm transformer encoder.

Reference model:
  x = emb[seq] + pes                                  # [B,S,D] = [4,512,1024]
  6x: x = BN(x + attn(x)); x = BN(x + ffn(x))
  BN = per-channel batch stats over (B,S), eps=1e-3.

Sharding: tensor-parallel across 8 NeuronCores. Each core owns H/8=2 heads
(QKV out / Wo in slices) and DF/8=512 FFN hidden units. After Wo and after
W2 an fp32 AllReduce combines partial [D, T] outputs; the residual x/8 is
folded into each partial via an extra (1/8)*I matmul so the AllReduce
directly yields x + sublayer(x). bo/b2 biases cancel inside BN and are
dropped. BatchNorm is computed redundantly on every core, keeping the
program SPMD-uniform (no rank-dependent addressing anywhere).

Activation layout: transposed. x^T lives in SBUF as [128 part, 8 dtile,
2048 tok] so natural-layout weights serve directly as matmul lhsT
(stationary) and activations as rhs (moving); no per-layer activation
transposes. Attention per (batch, head): scores^T = K_h @ Q_h^T,
E = exp(scale*scores^T) (softmax max-subtraction skipped; scores are O(1)),
U^T = V_h^T @ E^T with column sums from a ones-row matmul, normalized by a
PE-broadcast reciprocal row. Matmuls run as float32r (full-rate fp32 PE
mode; plain fp32 is 4x slower).
"""

import os

import numpy as np

import concourse.bass as bass
import concourse.mybir as mybir
import concourse.tile as tile
from concourse import bacc
from concourse.bass import ts
from concourse.masks import make_identity

# ---------------------------------------------------------------- dims
V, D, L, H, B, S = 32000, 1024, 6, 16, 4, 512
HD = D // H            # 64
DF = 4 * D             # 4096
EPS = 1e-3
NC = 8                 # cores
T = B * S              # 2048 tokens
P = 128                # partitions
DT = D // P            # 8 d-tiles
TT = T // P            # 16 token tiles
CH = 512               # token chunk (matmul N)
NCH = T // CH          # 4 chunks
HPC = H // NC          # heads per core = 2
DSH = HPC * HD         # qkv out shard = 128
FSH = DF // NC         # ffn hidden shard = 512
FMT = FSH // P         # ffn1 m-tiles = 4
KL = FSH // P          # ffn2 k-tiles = 4

f32 = mybir.dt.float32
f16 = mybir.dt.float16
f32r = mybir.dt.float32r
i16 = mybir.dt.int16
AF = mybir.ActivationFunctionType
ALU = mybir.AluOpType

REPLICAS = [list(range(NC))]

N_LAYERS = int(os.environ.get("TRN_KERNEL_LAYERS", str(L)))
DEBUG_TAPS = os.environ.get("TRN_KERNEL_DEBUG", "0") == "1"

GATHER_QUEUES = int(os.environ.get("TRN_GATHER_QUEUES", "1"))


def _r(ap):
    """view an fp32 AP as float32r for full-rate PE matmul"""
    return ap.bitcast(f32r)


def build_module(n_layers=None):
    if n_layers is None:
        n_layers = N_LAYERS
    nc = bacc.Bacc("TRN2", target_bir_lowering=False, debug=False,
                   num_devices=NC)

    dt_ = nc.dram_tensor
    io = {
        "emb": dt_("emb", [V, D], f32, kind="ExternalInput").ap(),
        "idx": dt_("idx", [16, T // 16], i16, kind="ExternalInput").ap(),
        "pesT": dt_("pesT", [D, S], f32, kind="ExternalInput").ap(),
        "wq": dt_("wq", [L, D, DSH], f32, kind="ExternalInput").ap(),
        "wk": dt_("wk", [L, D, DSH], f32, kind="ExternalInput").ap(),
        "wv": dt_("wv", [L, D, DSH], f32, kind="ExternalInput").ap(),
        "wo": dt_("wo", [L, DSH, D], f32, kind="ExternalInput").ap(),
        "w1": dt_("w1", [L, D, FSH], f32, kind="ExternalInput").ap(),
        "w2": dt_("w2", [L, FSH, D], f32, kind="ExternalInput").ap(),
        "bq": dt_("bq", [L, DSH], f32, kind="ExternalInput").ap(),
        "bk": dt_("bk", [L, DSH], f32, kind="ExternalInput").ap(),
        "bv": dt_("bv", [L, DSH], f32, kind="ExternalInput").ap(),
        "b1": dt_("b1", [L, FSH], f32, kind="ExternalInput").ap(),
        "g1": dt_("g1", [L, D], f32, kind="ExternalInput").ap(),
        "be1": dt_("be1", [L, D], f32, kind="ExternalInput").ap(),
        "g2": dt_("g2", [L, D], f32, kind="ExternalInput").ap(),
        "be2": dt_("be2", [L, D], f32, kind="ExternalInput").ap(),
        "out": dt_("out", [D, T], f32, kind="ExternalOutput").ap(),
    }
    if DEBUG_TAPS:
        for nm, shp in [("dbg_x", [D, T]), ("dbg_q", [P, T]), ("dbg_k", [P, T]),
                        ("dbg_v", [P, TT * DSH]), ("dbg_attn", [P, T]),
                        ("dbg_y1", [D, T]), ("dbg_x2", [D, T])]:
            io[nm] = dt_(nm, shp, f32, kind="ExternalOutput").ap()

    with tile.TileContext(nc) as tc:
        _build(tc, n_layers, io)
    nc.compile()
    return nc


def _build(tc, n_layers, io):
    from contextlib import ExitStack
    nc = tc.nc
    att_scale = 1.0 / np.sqrt(HD)

    # ------------------------------------------------ pools
    st = ExitStack()
    persist = st.enter_context(tc.tile_pool(name="persist", bufs=1))
    wpool = st.enter_context(tc.tile_pool(name="wpool", bufs=1))   # W1/W2
    wqkv = st.enter_context(tc.tile_pool(name="wqkv", bufs=1))     # Wq/Wk/Wv/Wo
    small = st.enter_context(tc.tile_pool(name="small", bufs=2))   # biases/stats
    tok8k = st.enter_context(tc.tile_pool(name="tok8k", bufs=2))   # [128, T]
    e512 = st.enter_context(tc.tile_pool(name="e512", bufs=6))     # [128, CH]
    htp = st.enter_context(tc.tile_pool(name="htp", bufs=2))       # [128,FMT,CH]
    ps = st.enter_context(tc.tile_pool(name="ps", bufs=5, space="PSUM"))
    pst = st.enter_context(tc.tile_pool(name="pst", bufs=2, space="PSUM"))
    drin = st.enter_context(tc.tile_pool(name="drin", bufs=2, space="DRAM"))
    drout = st.enter_context(tc.tile_pool(name="drout", bufs=2, space="DRAM"))

    # ------------------------------------------------ persistent tiles
    xbuf = persist.tile([P, DT, T], f32, name="xbuf")      # x / x2 (fp32)
    qT = persist.tile([P, T], f32, name="qT")              # Q^T shard
    kT = persist.tile([P, T], f32, name="kT")              # K^T shard
    vsb = persist.tile([P, TT, 2 * (HD + 1)], f32, name="vsb")  # [V|1|V|1]
    ident = persist.tile([P, P], f32, name="ident")
    eye8 = persist.tile([P, P], f32, name="eye8")
    onesP64 = persist.tile([P, 64], f32, name="onesP64")
    attnTA = persist.tile([HD, T], f32, name="attnTA")     # head-0 attn^T
    attnTB = persist.tile([HD, T], f32, name="attnTB")     # head-1 attn^T
    idxs = persist.tile([P, T // 16], i16, name="idxs")

    make_identity(nc, ident[:])
    nc.scalar.mul(_r(eye8[:]), ident[:], 1.0 / NC)         # (1/8) * I
    nc.vector.memset(onesP64[:], 1.0)
    nc.scalar.activation(_r(vsb[:, :, HD:HD + 1]), ident[:, 0:TT].unsqueeze(-1),
                         AF.Identity, bias=1.0, scale=0.0)
    nc.scalar.activation(_r(vsb[:, :, 2 * HD + 1:]), ident[:, 0:TT].unsqueeze(-1),
                         AF.Identity, bias=1.0, scale=0.0)
    # indices wrapped in 16 partitions, replicated into all 8 Q7-core stripes
    for r_ in range(P // 16):
        nc.sync.dma_start(idxs[16 * r_:16 * (r_ + 1), :], io["idx"])

    # ---------------------------------------- embedding: x^T = (emb[seq])^T + pes^T
    pes_lo = htp.tile([P, FMT, CH], f32, tag="ht", name="pes_lo")
    pes_hi = htp.tile([P, FMT, CH], f32, tag="ht", name="pes_hi")
    pes_r = io["pesT"].rearrange("(k p) s -> p k s", p=P)
    nc.sync.dma_start(pes_lo[:], pes_r[:, 0:4, :])
    nc.sync.dma_start(pes_hi[:], pes_r[:, 4:8, :])

    for half in range(TT // 2):  # gather 2 token-tiles (256 rows) at a time
        gtile = tok8k.tile([P, 2, D], f32, tag="tok", name=f"gt{half}")
        nc.gpsimd.dma_gather(
            out_ap=gtile[:],
            in_ap=io["emb"],
            idxs_ap=idxs[:, half * 16:(half + 1) * 16],
            num_idxs=2 * P,
            num_idxs_reg=2 * P,
            elem_size=D,
            queue_num=half % GATHER_QUEUES,
        )
        for j in range(2):
            t = half * 2 + j            # token tile index
            pos_t = t % (S // P)        # position tile within the batch
            for k in range(DT):
                ptile = pst.tile([P, P], f32, tag="tp", name=f"tp{t}_{k}")
                nc.tensor.transpose(ptile[:], gtile[:, j, ts(k, P)], ident[:])
                pes_src = pes_lo if k < 4 else pes_hi
                nc.vector.tensor_tensor(
                    out=_r(xbuf[:, k, ts(t, P)]),
                    in0=ptile[:],
                    in1=pes_src[:, k % 4, ts(pos_t, P)],
                    op=ALU.add,
                )

    if DEBUG_TAPS:
        nc.sync.dma_start(io["dbg_x"].rearrange("(k p) t -> p k t", p=P), xbuf[:])

    # ---------------------------------------- batchnorm (redundant, full-D)
    def batchnorm(lbl, arout_t, g_sb, be_sb):
        ysum = small.tile([P, DT], f32, tag="ysum", name=f"ysum{lbl}")
        sqp = small.tile([P, DT, NCH], f32, tag="sqp", name=f"sqp{lbl}")
        for k in range(DT):
            yt = tok8k.tile([P, T], f32, tag="tok", name=f"yt{lbl}_{k}")
            nc.sync.dma_start(yt[:], arout_t[ts(k, P), :])
            nc.vector.reduce_sum(out=ysum[:, k:k + 1], in_=yt[:],
                                 axis=mybir.AxisListType.X)
            for c in range(NCH):
                scr = e512.tile([P, CH], f32, tag="e", name=f"sq{lbl}_{k}_{c}")
                nc.scalar.activation(scr[:], yt[:, ts(c, CH)], AF.Square,
                                     accum_out=sqp[:, k, c:c + 1])
        sq = small.tile([P, DT], f32, tag="sq", name=f"sq{lbl}")
        nc.vector.reduce_sum(out=sq[:], in_=sqp[:], axis=mybir.AxisListType.X)
        mean = small.tile([P, DT], f32, tag="mean", name=f"mean{lbl}")
        nc.vector.tensor_scalar_mul(mean[:], ysum[:], 1.0 / T)
        msq = small.tile([P, DT], f32, tag="msq", name=f"msq{lbl}")
        nc.vector.tensor_tensor(out=msq[:], in0=mean[:], in1=mean[:], op=ALU.mult)
        veps = small.tile([P, DT], f32, tag="veps", name=f"veps{lbl}")
        # veps = sq/T - mean^2 + EPS
        nc.vector.scalar_tensor_tensor(out=veps[:], in0=sq[:], scalar=1.0 / T,
                                       in1=msq[:], op0=ALU.mult, op1=ALU.subtract)
        nc.vector.tensor_scalar_add(veps[:], veps[:], EPS)
        rec = small.tile([P, DT], f32, tag="rec", name=f"rec{lbl}")
        nc.vector.reciprocal(rec[:], veps[:])
        rstd = small.tile([P, DT], f32, tag="rstd", name=f"rstd{lbl}")
        nc.scalar.sqrt(rstd[:], rec[:])
        sc = small.tile([P, DT], f32, tag="sc", name=f"sc{lbl}")
        nc.vector.tensor_tensor(out=sc[:], in0=g_sb[:], in1=rstd[:], op=ALU.mult)
        sh = small.tile([P, DT], f32, tag="sh", name=f"sh{lbl}")
        nc.vector.tensor_tensor(out=sh[:], in0=mean[:], in1=sc[:], op=ALU.mult)
        nc.vector.tensor_tensor(out=sh[:], in0=be_sb[:], in1=sh[:], op=ALU.subtract)
        for k in range(DT):
            yt = tok8k.tile([P, T], f32, tag="tok", name=f"ya{lbl}_{k}")
            nc.sync.dma_start(yt[:], arout_t[ts(k, P), :])
            nc.scalar.activation(_r(xbuf[:, k, :]), yt[:], AF.Identity,
                                 bias=sh[:, k:k + 1], scale=sc[:, k:k + 1])

    # ---------------------------------------- layers
    for l in range(n_layers):
        # ---- layer weights/params to SBUF
        wq_sb = wqkv.tile([P, DT, DSH], f32, tag="wq", name=f"wq{l}")
        wk_sb = wqkv.tile([P, DT, DSH], f32, tag="wk", name=f"wk{l}")
        wv_sb = wqkv.tile([P, DT, DSH], f32, tag="wv", name=f"wv{l}")
        wo_sbA = wqkv.tile([HD, D], f32, tag="woA", name=f"woA{l}")
        wo_sbB = wqkv.tile([HD, D], f32, tag="woB", name=f"woB{l}")
        w1_sb = wpool.tile([P, DT, FSH], f32, tag="w1", name=f"w1{l}")
        w2_sb = wpool.tile([P, KL, D], f32, tag="w2", name=f"w2{l}")
        nc.sync.dma_start(_r(wq_sb[:]), _r(io["wq"][l].rearrange("(k p) m -> p k m", p=P)))
        nc.sync.dma_start(_r(wk_sb[:]), _r(io["wk"][l].rearrange("(k p) m -> p k m", p=P)))
        nc.sync.dma_start(_r(wv_sb[:]), _r(io["wv"][l].rearrange("(k p) m -> p k m", p=P)))
        nc.sync.dma_start(_r(wo_sbA[:]), _r(io["wo"][l][0:HD, :]))
        nc.sync.dma_start(_r(wo_sbB[:]), _r(io["wo"][l][HD:2 * HD, :]))
        nc.sync.dma_start(_r(w1_sb[:]), _r(io["w1"][l].rearrange("(k p) m -> p k m", p=P)))
        nc.sync.dma_start(_r(w2_sb[:]), _r(io["w2"][l].rearrange("(k p) m -> p k m", p=P)))

        bq_sb = small.tile([P, 1], f32, tag="bq", name=f"bq{l}")
        bk_sb = small.tile([P, 1], f32, tag="bk", name=f"bk{l}")
        bv_sb = small.tile([P, 1], f32, tag="bv", name=f"bv{l}")
        b1_sb = small.tile([P, FMT], f32, tag="b1", name=f"b1{l}")
        nc.sync.dma_start(bq_sb[:], io["bq"][l].rearrange("(p o) -> p o", o=1))
        nc.sync.dma_start(bk_sb[:], io["bk"][l].rearrange("(p o) -> p o", o=1))
        nc.sync.dma_start(bv_sb[:], io["bv"][l].rearrange("(p o) -> p o", o=1))
        nc.sync.dma_start(b1_sb[:], io["b1"][l].rearrange("(m p) -> p m", p=P))

        g1_sb = small.tile([P, DT], f32, tag="g1", name=f"g1{l}")
        be1_sb = small.tile([P, DT], f32, tag="be1", name=f"be1{l}")
        g2_sb = small.tile([P, DT], f32, tag="g2", name=f"g2{l}")
        be2_sb = small.tile([P, DT], f32, tag="be2", name=f"be2{l}")
        nc.sync.dma_start(g1_sb[:], io["g1"][l].rearrange("(k p) -> p k", p=P))
        nc.sync.dma_start(be1_sb[:], io["be1"][l].rearrange("(k p) -> p k", p=P))
        nc.sync.dma_start(g2_sb[:], io["g2"][l].rearrange("(k p) -> p k", p=P))
        nc.sync.dma_start(be2_sb[:], io["be2"][l].rearrange("(k p) -> p k", p=P))

        # ---- QKV projections (shard): Q^T/K^T/V^T = W_shard^T @ x^T
        vT = tok8k.tile([P, T], f32, tag="tok", name=f"vT{l}")
        for c in range(NCH):
            psq = ps.tile([P, CH], f32, tag="mm", name=f"psq{l}_{c}")
            psk = ps.tile([P, CH], f32, tag="mm", name=f"psk{l}_{c}")
            psv = ps.tile([P, CH], f32, tag="mm", name=f"psv{l}_{c}")
            for k in range(DT):
                fl, ll = (k == 0), (k == DT - 1)
                rhs = _r(xbuf[:, k, ts(c, CH)])
                nc.tensor.matmul(psq[:], _r(wq_sb[:, k, :]), rhs, start=fl, stop=ll)
                nc.tensor.matmul(psk[:], _r(wk_sb[:, k, :]), rhs, start=fl, stop=ll)
                nc.tensor.matmul(psv[:], _r(wv_sb[:, k, :]), rhs, start=fl, stop=ll)
            nc.scalar.activation(_r(qT[:, ts(c, CH)]), psq[:], AF.Identity, bias=bq_sb[:])
            nc.scalar.activation(_r(kT[:, ts(c, CH)]), psk[:], AF.Identity, bias=bk_sb[:])
            nc.scalar.activation(vT[:, ts(c, CH)], psv[:], AF.Identity, bias=bv_sb[:])

        # ---- V^T -> V (token-partition layout) via PE transposes
        for t in range(TT):
            ptile = pst.tile([P, P], f32, tag="tp", name=f"vt{l}_{t}")
            nc.tensor.transpose(ptile[:], vT[:, ts(t, P)], ident[:])
            nc.vector.tensor_copy(
                _r(vsb[:, t, :].rearrange("p (h x) -> p h x", h=2)[:, :, 0:HD]),
                ptile[:].rearrange("p (h x) -> p h x", h=2))

        # ---- attention: per head all-f32r at PSUM base 0; softmax sums
        # fused into the U matmul via the ones-column appended to V.
        for b in range(B):
            for h, attnT_h in enumerate([attnTA, attnTB]):
                hp = h * HD
                vof = h * (HD + 1)
                ets = []
                for sk in range(B):
                    pss = ps.tile([P, CH], f32, tag="mm",
                                  name=f"pss{l}_{b}_{h}_{sk}")
                    nc.tensor.matmul(
                        pss[:],
                        _r(kT[hp:hp + HD, b * CH + sk * P:b * CH + (sk + 1) * P]),
                        _r(qT[hp:hp + HD, ts(b, CH)]),
                        start=True, stop=True)
                    et = e512.tile([P, CH], f32, tag="e",
                                   name=f"et{l}_{b}_{h}_{sk}")
                    nc.scalar.activation(_r(et[:]), pss[:], AF.Exp, scale=att_scale)
                    ets.append(et)
                psu = ps.tile([P, CH], f32, tag="mm", name=f"psu{l}_{b}_{h}")
                for sk in range(B):
                    nc.tensor.matmul(psu[0:HD + 1, :],
                                     _r(vsb[:, b * 4 + sk, vof:vof + HD + 1]),
                                     _r(ets[sk][:]),
                                     start=(sk == 0), stop=(sk == B - 1))
                rsb = e512.tile([P, CH], f32, tag="e", name=f"rsb{l}_{b}_{h}")
                nc.vector.reciprocal(rsb[HD:HD + 1, :], psu[HD:HD + 1, :])
                psr = ps.tile([P, CH], f32, tag="mm", name=f"psr{l}_{b}_{h}")
                nc.tensor.matmul(psr[0:HD, :], onesP64[HD:HD + 1, :],
                                 rsb[HD:HD + 1, :], start=True, stop=True)
                usb = e512.tile([P, CH], f32, tag="e", name=f"usb{l}_{b}_{h}")
                nc.scalar.copy(usb[0:HD, :], psu[0:HD, :])
                nc.vector.tensor_tensor(out=_r(attnT_h[:, ts(b, CH)]),
                                        in0=usb[0:HD, :],
                                        in1=psr[0:HD, :], op=ALU.mult)

        if DEBUG_TAPS and l == 0:
            nc.sync.dma_start(io["dbg_q"], qT[:])
            nc.sync.dma_start(io["dbg_k"], kT[:])
            nc.sync.dma_start(io["dbg_v"], vsb[:].rearrange("p a b -> p (a b)"))
            nc.sync.dma_start(io["dbg_attn"], attnTA[:].rearrange("p t -> p t"))

        # ---- Wo partial + residual/8 -> AllReduce
        arin1 = drin.tile([D, T], f32, tag="ari", name=f"ari1_{l}")
        arout1 = drout.tile([D, T], f32, tag="aro", addr_space="Shared",
                            name=f"aro1_{l}")
        for m in range(DT):
            for c in range(NCH):
                ps2 = ps.tile([P, CH], f32, tag="mm", name=f"pso{l}_{m}_{c}")
                nc.tensor.matmul(ps2[:], _r(wo_sbA[:, ts(m, P)]),
                                 _r(attnTA[:, ts(c, CH)]), start=True, stop=False)
                nc.tensor.matmul(ps2[:], _r(wo_sbB[:, ts(m, P)]),
                                 _r(attnTB[:, ts(c, CH)]), start=False, stop=False)
                nc.tensor.matmul(ps2[:], _r(eye8[:]), _r(xbuf[:, m, ts(c, CH)]),
                                 start=False, stop=True)
                osb = e512.tile([P, CH], f32, tag="e", name=f"osb{l}_{m}_{c}")
                nc.vector.tensor_copy(osb[:], ps2[:])
                nc.sync.dma_start(arin1[ts(m, P), ts(c, CH)], osb[:])
        nc.gpsimd.collective_compute(
            "AllReduce", ALU.add, replica_groups=REPLICAS,
            ins=[arin1.opt()], outs=[arout1.opt()])

        if DEBUG_TAPS and l == 0:
            nc.sync.dma_start(io["dbg_y1"], arout1)

        # ---- BN1 -> x2 into xbuf
        batchnorm(f"a{l}", arout1, g1_sb, be1_sb)
        if DEBUG_TAPS and l == 0:
            nc.sync.dma_start(io["dbg_x2"].rearrange("(k p) t -> p k t", p=P),
                              xbuf[:])

        # ---- FFN (chunk-major so h^T is chunk-resident) + residual/8 -> AR
        arin2 = drin.tile([D, T], f32, tag="ari", name=f"ari2_{l}")
        arout2 = drout.tile([D, T], f32, tag="aro", addr_space="Shared",
                            name=f"aro2_{l}")
        for c in range(NCH):
            ht = htp.tile([P, FMT, CH], f32, tag="ht", name=f"ht{l}_{c}")
            for m in range(FMT):
                ps1 = ps.tile([P, CH], f32, tag="mm", name=f"ps1{l}_{c}_{m}")
                for k in range(DT):
                    nc.tensor.matmul(ps1[:], _r(w1_sb[:, k, ts(m, P)]),
                                     _r(xbuf[:, k, ts(c, CH)]),
                                     start=(k == 0), stop=(k == DT - 1))
                nc.scalar.activation(_r(ht[:, m, :]), ps1[:], AF.Relu,
                                     bias=b1_sb[:, m:m + 1])
            for m in range(DT):
                ps2 = ps.tile([P, CH], f32, tag="mm", name=f"ps2{l}_{c}_{m}")
                for k in range(KL):
                    nc.tensor.matmul(ps2[:], _r(w2_sb[:, k, ts(m, P)]),
                                     _r(ht[:, k, :]), start=(k == 0), stop=False)
                nc.tensor.matmul(ps2[:], _r(eye8[:]), _r(xbuf[:, m, ts(c, CH)]),
                                 start=False, stop=True)
                osb = e512.tile([P, CH], f32, tag="e", name=f"fsb{l}_{c}_{m}")
                nc.vector.tensor_copy(osb[:], ps2[:])
                nc.sync.dma_start(arin2[ts(m, P), ts(c, CH)], osb[:])
        nc.gpsimd.collective_compute(
            "AllReduce", ALU.add, replica_groups=REPLICAS,
            ins=[arin2.opt()], outs=[arout2.opt()])

        # ---- BN2 -> x(l+1) into xbuf
        batchnorm(f"f{l}", arout2, g2_sb, be2_sb)

    # ---------------------------------------- output x^T -> [D, T]
    nc.sync.dma_start(io["out"].rearrange("(k p) t -> p k t", p=P), xbuf[:])
    st.close()


# ================================================================ host side

def make_in_maps(inputs):
    f = lambda a: np.ascontiguousarray(np.asarray(a), dtype=np.float32)
    seq = np.asarray(inputs["sequence"]).reshape(-1).astype(np.int16)
    idx = np.ascontiguousarray(seq.reshape(T // 16, 16).T)     # [16, T//16]
    emb = f(inputs["emb"])
    pesT = np.ascontiguousarray(f(inputs["pes"]).T)            # [D, S]
    Wq, Wk, Wv = f(inputs["Wq"]), f(inputs["Wk"]), f(inputs["Wv"])
    Wo, W1, W2 = f(inputs["Wo"]), f(inputs["W1"]), f(inputs["W2"])
    bq, bk, bv = f(inputs["bq"]), f(inputs["bk"]), f(inputs["bv"])
    b1 = f(inputs["b1"])
    g1, be1 = f(inputs["g1"]), f(inputs["be1"])
    g2, be2 = f(inputs["g2"]), f(inputs["be2"])

    in_maps = []
    for c in range(NC):
        ds_ = slice(c * DSH, (c + 1) * DSH)
        fs_ = slice(c * FSH, (c + 1) * FSH)
        in_maps.append({
            "emb": emb,
            "idx": idx,
            "pesT": pesT,
            "wq": np.ascontiguousarray(Wq[:, :, ds_]),
            "wk": np.ascontiguousarray(Wk[:, :, ds_]),
            "wv": np.ascontiguousarray(Wv[:, :, ds_]),
            "wo": np.ascontiguousarray(Wo[:, ds_, :]),
            "w1": np.ascontiguousarray(W1[:, :, fs_]),
            "w2": np.ascontiguousarray(W2[:, fs_, :]),
            "bq": np.ascontiguousarray(bq[:, ds_]),
            "bk": np.ascontiguousarray(bk[:, ds_]),
            "bv": np.ascontiguousarray(bv[:, ds_]),
            "b1": np.ascontiguousarray(b1[:, fs_]),
            "g1": g1, "be1": be1, "g2": g2, "be2": be2,
        })
    return in_maps


_CACHE = {}


def _get_module():
    if "nc" not in _CACHE:
        _CACHE["nc"] = build_module()
    return _CACHE["nc"]


def kernel(**inputs):
    from concourse import bass_utils
    nc = _get_module()
    in_maps = make_in_maps(inputs)
    res = bass_utils.run_bass_kernel_spmd(nc, in_maps, list(range(NC)))
    o = np.asarray(res.results[0]["out"])                  # [D, T]
    return np.ascontiguousarray(o.T).reshape(B, S, D).astype(np.float32)



# revision 28
# speedup vs baseline: 1.9005x; 1.9005x over previous
"""Trainium2 Bass kernel for a 6-layer post-BatchNorm transformer encoder.

Reference model:
  x = emb[seq] + pes                                  # [B,S,D] = [4,512,1024]
  6x: x = BN(x + attn(x)); x = BN(x + ffn(x))
  BN = per-channel batch stats over (B,S), eps=1e-3.

Sharding: token-sharded data parallel across 8 NeuronCores. Core c owns the
256 contiguous tokens [256c, 256c+256) = batch c//2, sequence half c%2. All
matmuls are local full-width (every core streams the full bf16 weights from
HBM in 1MB chunks through a ring); residual adds and BatchNorm application
are local. Cross-core communication per layer:
  - one pair AllGather (cores 2b,2b+1) of K^T and V (bf16, 1MB in / 2MB out)
    so attention sees the full 512-key sequence of its batch,
  - two 8KB AllGathers of per-core BN partial sums/sumsq (the only global
    coupling BatchNorm actually needs).
bk/bv/bo/b2 biases cancel mathematically (bk/bv through softmax rows summing
to 1, bo/b2 inside BN mean subtraction) and are dropped; bq and b1 are kept.

Numerics: all matmuls in bf16 (fp32 PSUM accumulation); x kept in fp32
master + bf16 matmul copy; BN statistics in fp32.

Layout: activations transposed [128 part, dtile, tokens]; weights natural
[Din, Dout] serve as lhsT. Attention per head pair packs the two heads at
partition bases 0/64 (row-group concurrency on the PE). Softmax sums ride
as a ones-column in the even head's V (PSUM rows 64) and a separate
ones-row matmul into PSUM row 96 for the odd head; the odd head's U lands
at PSUM rows 64:128 via output col-group 64, so every downstream copy is
partition-aligned. Embedding gather uses dma_gather(transpose=True), which
lands rows directly in the transposed layout.
"""

import os

import numpy as np

import concourse.bass as bass
import concourse.mybir as mybir
import concourse.tile as tile
from concourse import bacc
from concourse.bass import ts

# ---------------------------------------------------------------- dims
V, D, L, H, B, S = 32000, 1024, 6, 16, 4, 512
HD = D // H            # 64
DF = 4 * D             # 4096
EPS = 1e-3
NC = 8                 # cores
T = B * S              # 2048 tokens total
TL = T // NC           # 256 tokens per core
P = 128                # partitions
DT = D // P            # 8 d-tiles
FMT = DF // P          # 32 ffn1 m-tiles
SK = S // P            # 4 key chunks per batch

f32 = mybir.dt.float32
bf16 = mybir.dt.bfloat16
i16 = mybir.dt.int16
AF = mybir.ActivationFunctionType
ALU = mybir.AluOpType

ALLGRP = [list(range(NC))]
KVGRP = [[2 * b, 2 * b + 1] for b in range(B)]

N_LAYERS = int(os.environ.get("TRN_KERNEL_LAYERS", str(L)))
DEBUG_TAPS = os.environ.get("TRN_KERNEL_DEBUG", "0") == "1"
QSPLIT = os.environ.get("TRN_QSPLIT", "1") == "1"
FASTRECIP = os.environ.get("TRN_FASTRECIP", "1") == "1"
USE_TTR = os.environ.get("TRN_TTR", "0") == "1"

KVB = TL * D           # elements of one K^T or V block in the kv exchange


def build_module(n_layers=None):
    if n_layers is None:
        n_layers = N_LAYERS
    nc = bacc.Bacc("TRN2", target_bir_lowering=False, debug=False,
                   num_devices=NC)

    dt_ = nc.dram_tensor
    io = {
        "emb": dt_("emb", [V, D], bf16, kind="ExternalInput").ap(),
        "idx": dt_("idx", [16, TL // 16], i16, kind="ExternalInput").ap(),
        "pesT": dt_("pesT", [D, TL], f32, kind="ExternalInput").ap(),
        "wq": dt_("wq", [L, D, D], bf16, kind="ExternalInput").ap(),
        "wk": dt_("wk", [L, D, D], bf16, kind="ExternalInput").ap(),
        "wv": dt_("wv", [L, D, D], bf16, kind="ExternalInput").ap(),
        "wo": dt_("wo", [L, D, D], bf16, kind="ExternalInput").ap(),
        "w1": dt_("w1", [L, D, DF], bf16, kind="ExternalInput").ap(),
        "w2": dt_("w2", [L, DF, D], bf16, kind="ExternalInput").ap(),
        "bq": dt_("bq", [L, D], f32, kind="ExternalInput").ap(),
        "b1": dt_("b1", [L, DF], f32, kind="ExternalInput").ap(),
        "g1": dt_("g1", [L, D], f32, kind="ExternalInput").ap(),
        "be1": dt_("be1", [L, D], f32, kind="ExternalInput").ap(),
        "g2": dt_("g2", [L, D], f32, kind="ExternalInput").ap(),
        "be2": dt_("be2", [L, D], f32, kind="ExternalInput").ap(),
        "out": dt_("out", [D, TL], f32, kind="ExternalOutput").ap(),
    }
    if DEBUG_TAPS:
        for nm, shp in [("dbg_x", [D, TL]), ("dbg_y1", [D, TL]),
                        ("dbg_x2", [D, TL]), ("dbg_y2", [D, TL])]:
            io[nm] = dt_(nm, shp, f32, kind="ExternalOutput").ap()
        for nm, shp in [("dbg_q", [D, TL]), ("dbg_k", [D, S]),
                        ("dbg_vsb", [P, SK * H * (HD + 1)]),
                        ("dbg_attnT", [D, TL]), ("dbg_h", [DF, TL])]:
            io[nm] = dt_(nm, shp, bf16, kind="ExternalOutput").ap()

    with tile.TileContext(nc) as tc:
        _build(tc, n_layers, io)
    nc.compile()
    return nc


def _build(tc, n_layers, io):
    from contextlib import ExitStack
    nc = tc.nc
    att_scale = 1.0 / np.sqrt(HD)
    dmae = nc.scalar if QSPLIT else nc.sync

    st_ = ExitStack()
    persist = st_.enter_context(tc.tile_pool(name="persist", bufs=1))
    wpool = st_.enter_context(tc.tile_pool(name="wpool", bufs=8))
    ppool = st_.enter_context(tc.tile_pool(name="ppool", bufs=2))
    epool = st_.enter_context(tc.tile_pool(name="epool", bufs=10))
    spool = st_.enter_context(tc.tile_pool(name="spool", bufs=2))
    ps = st_.enter_context(tc.tile_pool(name="ps", bufs=3, space="PSUM"))
    drin = st_.enter_context(tc.tile_pool(name="drin", bufs=2, space="DRAM"))
    drout = st_.enter_context(tc.tile_pool(name="drout", bufs=2, space="DRAM"))

    # ---------------- persistent tiles
    xf32a = persist.tile([P, DT, TL], f32, name="xf32a")
    xf32b = persist.tile([P, DT, TL], f32, name="xf32b")
    xb16 = persist.tile([P, DT, TL], bf16, name="xb16")
    qT = persist.tile([P, DT, TL], bf16, name="qT")
    kloc = persist.tile([P, DT, TL], bf16, name="kloc")
    vloc = persist.tile([P, 2, D], bf16, name="vloc")
    kT = persist.tile([P, DT, S], bf16, name="kT")
    vsb = persist.tile([P, SK, H, HD + 1], bf16, name="vsb")
    attnT = persist.tile([P, DT, TL], bf16, name="attnT")
    ht = persist.tile([P, FMT, TL], bf16, name="ht")
    onesb = persist.tile([P, P], bf16, name="onesb")
    idxs = persist.tile([P, TL // 16], i16, name="idxs")

    nc.vector.memset(onesb[:], 1.0)
    nc.vector.memset(vsb[:, :, :, HD:HD + 1], 1.0)
    for r_ in range(P // 16):
        nc.sync.dma_start(idxs[16 * r_:16 * (r_ + 1), :], io["idx"])

    # ---------------- embedding: x^T = (emb[seq])^T + pes^T
    pes_sb = spool.tile([P, DT, TL], f32, tag="pes", bufs=1, name="pes_sb")
    dmae.dma_start(pes_sb[:], io["pesT"].rearrange("(k p) t -> p k t", p=P))
    gt = spool.tile([P, DT, TL], bf16, tag="gt", bufs=1, name="gt")
    nc.gpsimd.dma_gather(
        out_ap=gt[:], in_ap=io["emb"], idxs_ap=idxs[:],
        num_idxs=TL, num_idxs_reg=TL, elem_size=D, transpose=True)
    for k in range(DT):
        nc.vector.tensor_tensor(out=xf32a[:, k, :], in0=gt[:, k, :],
                                in1=pes_sb[:, k, :], op=ALU.add)
    nc.vector.tensor_copy(xb16[:], xf32a[:])

    if DEBUG_TAPS:
        nc.sync.dma_start(io["dbg_x"].rearrange("(k p) t -> p k t", p=P),
                          xf32a[:])

    xcur = xf32a
    xnxt = xf32b

    # ---------------- per-layer param loads (small)
    def load_params(l):
        bq_sb = ppool.tile([P, DT], f32, tag="bq", name=f"bq{l}")
        b1_sb = ppool.tile([P, FMT], f32, tag="b1", name=f"b1{l}")
        g1_sb = ppool.tile([P, DT], f32, tag="g1", name=f"g1{l}")
        be1_sb = ppool.tile([P, DT], f32, tag="be1", name=f"be1{l}")
        g2_sb = ppool.tile([P, DT], f32, tag="g2", name=f"g2{l}")
        be2_sb = ppool.tile([P, DT], f32, tag="be2", name=f"be2{l}")
        dmae.dma_start(bq_sb[:], io["bq"][l].rearrange("(m p) -> p m", p=P))
        dmae.dma_start(b1_sb[:], io["b1"][l].rearrange("(m p) -> p m", p=P))
        dmae.dma_start(g1_sb[:], io["g1"][l].rearrange("(k p) -> p k", p=P))
        dmae.dma_start(be1_sb[:], io["be1"][l].rearrange("(k p) -> p k", p=P))
        dmae.dma_start(g2_sb[:], io["g2"][l].rearrange("(k p) -> p k", p=P))
        dmae.dma_start(be2_sb[:], io["be2"][l].rearrange("(k p) -> p k", p=P))
        return bq_sb, b1_sb, g1_sb, be1_sb, g2_sb, be2_sb

    # weight chunk loader: returns [P, DT, 512] (half the out-cols of a DxD
    # weight) or [P, 4, D] (4 k-tiles of w2)
    def wchunk(src_ap, l, nm):
        t = wpool.tile(list(src_ap.shape), bf16, tag="w", name=nm)
        nc.sync.dma_start(t[:], src_ap)
        return t

    # BN stats -> AllGather -> sc/sh
    def bn_reduce(lbl, stats, g_sb, be_sb):
        sti = drin.tile([P * 16], f32, tag="sti", name=f"sti{lbl}")
        sto = drout.tile([NC * P * 16], f32, tag="sto", addr_space="Shared",
                         name=f"sto{lbl}")
        dmae.dma_start(sti[:].rearrange("(p s) -> p s", p=P), stats[:])
        nc.gpsimd.collective_compute(
            "AllGather", ALU.bypass, replica_groups=ALLGRP,
            ins=[sti[:].opt()], outs=[sto[:].opt()])
        ld = spool.tile([P, 16, NC], f32, tag="ld", name=f"ld{lbl}")
        dmae.dma_start(ld[:], sto[:].rearrange("(r p s) -> p s r", p=P, s=16))
        tot = spool.tile([P, 16], f32, tag="tot", name=f"tot{lbl}")
        nc.vector.reduce_sum(out=tot[:], in_=ld[:], axis=mybir.AxisListType.X)
        mean = spool.tile([P, DT], f32, tag="mean", name=f"mean{lbl}")
        nc.vector.tensor_scalar_mul(mean[:], tot[:, 0:DT], 1.0 / T)
        msq = spool.tile([P, DT], f32, tag="msq", name=f"msq{lbl}")
        nc.vector.tensor_tensor(out=msq[:], in0=mean[:], in1=mean[:], op=ALU.mult)
        veps = spool.tile([P, DT], f32, tag="veps", name=f"veps{lbl}")
        nc.vector.scalar_tensor_tensor(out=veps[:], in0=tot[:, DT:16],
                                       scalar=1.0 / T, in1=msq[:],
                                       op0=ALU.mult, op1=ALU.subtract)
        nc.vector.tensor_scalar_add(veps[:], veps[:], EPS)
        rec = spool.tile([P, DT], f32, tag="rec", name=f"rec{lbl}")
        nc.vector.reciprocal(rec[:], veps[:])
        rstd = spool.tile([P, DT], f32, tag="rstd", name=f"rstd{lbl}")
        nc.scalar.sqrt(rstd[:], rec[:])
        sc = spool.tile([P, DT], f32, tag="sc", name=f"sc{lbl}")
        nc.vector.tensor_tensor(out=sc[:], in0=g_sb[:], in1=rstd[:], op=ALU.mult)
        sh = spool.tile([P, DT], f32, tag="sh", name=f"sh{lbl}")
        nc.vector.tensor_tensor(out=sh[:], in0=mean[:], in1=sc[:], op=ALU.mult)
        nc.vector.tensor_tensor(out=sh[:], in0=be_sb[:], in1=sh[:], op=ALU.subtract)
        return sc, sh

    # ---------------- layers
    for l in range(n_layers):
        bq_sb, b1_sb, g1_sb, be1_sb, g2_sb, be2_sb = load_params(l)

        wk_r = io["wk"][l].rearrange("(k p) m -> p k m", p=P)
        wv_r = io["wv"][l].rearrange("(k p) m -> p k m", p=P)
        wq_r = io["wq"][l].rearrange("(k p) m -> p k m", p=P)
        wo_r = io["wo"][l].rearrange("(k p) m -> p k m", p=P)
        w1_r = io["w1"][l].rearrange("(k p) m -> p k m", p=P)
        w2_r = io["w2"][l].rearrange("(k p) m -> p k m", p=P)

        wk_ch = [wchunk(wk_r[:, :, ts(h, 512)], l, f"wk{l}_{h}") for h in range(2)]
        wv_ch = [wchunk(wv_r[:, :, ts(h, 512)], l, f"wv{l}_{h}") for h in range(2)]
        wq_ch = [wchunk(wq_r[:, :, ts(h, 512)], l, f"wq{l}_{h}") for h in range(2)]

        # ---- K projection (local tokens): K^T = Wk^T x^T
        for g in range(DT):
            psk = ps.tile([P, TL], f32, tag="mm", name=f"psk{l}_{g}")
            for k in range(DT):
                nc.tensor.matmul(psk[:], wk_ch[g // 4][:, k, ts(g % 4, P)],
                                 xb16[:, k, :], start=(k == 0), stop=(k == DT - 1))
            nc.vector.tensor_copy(kloc[:, g, :], psk[:])

        # ---- V projection, token-major: V = x W_v (x tiles stationary)
        for mt in range(2):
            for nb in range(4):
                psv = ps.tile([P, TL], f32, tag="mm", name=f"psv{l}_{mt}_{nb}")
                for k in range(DT):
                    nc.tensor.matmul(
                        psv[:], xb16[:, k, ts(mt, P)],
                        wv_ch[nb // 2][:, k, ts(nb % 2, 256)],
                        start=(k == 0), stop=(k == DT - 1))
                nc.vector.tensor_copy(vloc[:, mt, ts(nb, 256)], psv[:])

        # ---- ship local K/V, gather the batch pair's full K/V
        kvi = drin.tile([2 * KVB], bf16, tag="kvi", name=f"kvi{l}")
        kvo = drout.tile([2 * 2 * KVB], bf16, tag="kvo", name=f"kvo{l}")
        dmae.dma_start(
            kvi[0:KVB].rearrange("(tt p d) -> p tt d", tt=2, p=P), vloc[:])
        dmae.dma_start(
            kvi[KVB:2 * KVB].rearrange("(g p t) -> p g t", g=DT, p=P), kloc[:])
        nc.gpsimd.collective_compute(
            "AllGather", ALU.bypass, replica_groups=KVGRP,
            ins=[kvi[:].opt()], outs=[kvo[:].opt()])

        # ---- Q projection (overlaps the AllGather)
        for g in range(DT):
            psq = ps.tile([P, TL], f32, tag="mm", name=f"psq{l}_{g}")
            for k in range(DT):
                nc.tensor.matmul(psq[:], wq_ch[g // 4][:, k, ts(g % 4, P)],
                                 xb16[:, k, :], start=(k == 0), stop=(k == DT - 1))
            nc.vector.tensor_scalar_add(qT[:, g, :], psq[:], bq_sb[:, g:g + 1])

        # ---- land gathered K/V
        for hf in range(2):
            base = hf * 2 * KVB
            dmae.dma_start(
                kT[:, :, ts(hf, TL)],
                kvo[base + KVB:base + 2 * KVB].rearrange(
                    "(g p t) -> p g t", g=DT, p=P))
            for tt in range(2):
                vblk = kvo[base + tt * P * D:base + (tt + 1) * P * D]
                dmae.dma_start(
                    vsb[:, 2 * hf + tt, :, 0:HD],
                    vblk.rearrange("(p h hd) -> p h hd", p=P, h=H))

        if DEBUG_TAPS and l == 0:
            nc.sync.dma_start(io["dbg_q"].rearrange("(k p) t -> p k t", p=P), qT[:])
            nc.sync.dma_start(io["dbg_k"].rearrange("(k p) t -> p k t", p=P), kT[:])
            nc.sync.dma_start(
                io["dbg_vsb"].rearrange("p (a b c) -> p a b c", a=SK, b=H), vsb[:])

        wo_ch = [wchunk(wo_r[:, :, ts(h, 512)], l, f"wo{l}_{h}") for h in range(2)]

        # ---- attention. Phase A: all pairs' scores + exp (exp batched over
        # 2 key-chunks); PE streams scores back-to-back while scalar exps.
        eall = []
        for g in range(DT):
            epair = [[None, None], [None, None]]
            for kcb in range(2):
                sst = [None, None]
                for tw in range(2):
                    sst[tw] = ps.tile([P, 2, TL], f32, tag="s", bufs=2,
                                      name=f"pss{l}_{g}_{kcb}_{tw}")
                for j in range(2):
                    for tw in range(2):
                        hp = 64 * tw
                        nc.tensor.matmul(
                            sst[tw][:, j, :],
                            kT[hp:hp + HD, g, ts(2 * kcb + j, P)],
                            qT[hp:hp + HD, g, :], start=True, stop=True)
                for tw in range(2):
                    et = epool.tile([P, 2, TL], bf16, tag="e", bufs=18,
                                    name=f"et{l}_{g}_{kcb}_{tw}")
                    nc.scalar.activation(et[:], sst[tw][:], AF.Exp,
                                         scale=att_scale)
                    epair[tw][kcb] = et
            eall.append(epair)

        # Phase B1: per pair, U accumulation; unnormalized U and the sumexp
        # rows drain to SBUF so the reciprocals batch across all 8 pairs.
        serow = spool.tile([P, DT, TL], f32, tag="serow", name=f"serow{l}")
        rsall = spool.tile([P, DT, TL], bf16, tag="rsall", name=f"rsall{l}")
        usbs = []
        for g in range(DT):
            epair = eall[g]
            bankA = ps.tile([P, TL], f32, tag="u", name=f"bA{l}_{g}")
            bankB = ps.tile([P, TL], f32, tag="u", name=f"bB{l}_{g}")
            for kc in range(SK):
                fl, ll = (kc == 0), (kc == SK - 1)
                ee = epair[0][kc // 2][:, kc % 2, :]
                eo = epair[1][kc // 2][:, kc % 2, :]
                nc.tensor.matmul(bankA[0:HD + 1, :],
                                 vsb[:, kc, 2 * g, 0:HD + 1], ee,
                                 start=fl, stop=ll)
                nc.tensor.matmul(bankB[64:128, :],
                                 vsb[:, kc, 2 * g + 1, 0:HD], eo,
                                 start=fl, stop=ll)
                nc.tensor.matmul(bankA[96:97, :], onesb[:, 0:1], eo,
                                 start=fl, stop=ll, tile_position=(0, 96))
            usbE = epool.tile([P, TL], bf16, tag="usb", bufs=18,
                              name=f"uE{l}_{g}")
            usbO = epool.tile([P, TL], bf16, tag="usb", bufs=18,
                              name=f"uO{l}_{g}")
            nc.vector.tensor_copy(usbE[0:64, :], bankA[0:64, :])
            nc.vector.tensor_copy(usbO[64:128, :], bankB[64:128, :])
            nc.vector.tensor_copy(serow[HD:HD + 1, g, :], bankA[HD:HD + 1, :])
            nc.vector.tensor_copy(serow[96:97, g, :], bankA[96:97, :])
            usbs.append((usbE, usbO))
        with nc.allow_low_precision(reason="softmax 1/sumexp as bf16"):
            nc.vector.reciprocal(rsall[HD:HD + 1, :, :], serow[HD:HD + 1, :, :])
            nc.vector.reciprocal(rsall[96:97, :, :], serow[96:97, :, :])

        # Phase B2: broadcast 1/sumexp to the head's 64 partitions (PE) and
        # normalize into attnT.
        for g in range(DT):
            usbE, usbO = usbs[g]
            psr = ps.tile([P, TL], f32, tag="u", name=f"psr{l}_{g}")
            nc.tensor.matmul(psr[0:64, :], onesb[HD:HD + 1, 0:64],
                             rsall[HD:HD + 1, g, :], start=True, stop=True)
            nc.tensor.matmul(psr[64:128, :], onesb[96:97, 0:64],
                             rsall[96:97, g, :], start=True, stop=True,
                             tile_position=(96, 64))
            nc.vector.tensor_tensor(out=attnT[0:64, g, :], in0=usbE[0:64, :],
                                    in1=psr[0:64, :], op=ALU.mult)
            nc.vector.tensor_tensor(out=attnT[64:128, g, :], in0=usbO[64:128, :],
                                    in1=psr[64:128, :], op=ALU.mult)

        if DEBUG_TAPS and l == 0:
            nc.sync.dma_start(
                io["dbg_attnT"].rearrange("(k p) t -> p k t", p=P), attnT[:])

        w1_ch = [wchunk(w1_r[:, :, ts(h, 512)], l, f"w1{l}_{h}") for h in range(8)]

        # ---- Wo + residual -> y1 (fp32) with fused BN partial stats
        st1 = spool.tile([P, 16], f32, tag="st", name=f"st1_{l}")
        sqs = spool.tile([P, TL], f32, tag="sqs", name=f"sq1_{l}")
        for m in range(DT):
            pso = ps.tile([P, TL], f32, tag="mm", name=f"pso{l}_{m}")
            for k in range(DT):
                nc.tensor.matmul(pso[:], wo_ch[m // 4][:, k, ts(m % 4, P)],
                                 attnT[:, k, :], start=(k == 0), stop=(k == DT - 1))
            nc.vector.scalar_tensor_tensor(
                out=xnxt[:, m, :], in0=pso[:], scalar=1.0, in1=xcur[:, m, :],
                op0=ALU.mult, op1=ALU.add, accum_out=st1[:, m:m + 1])
            if USE_TTR:
                nc.vector.tensor_tensor_reduce(
                    out=sqs[:], in0=xnxt[:, m, :], in1=xnxt[:, m, :], scale=1.0,
                    scalar=0.0, op0=ALU.mult, op1=ALU.add,
                    accum_out=st1[:, DT + m:DT + m + 1])
            else:
                nc.scalar.activation(sqs[:], xnxt[:, m, :], AF.Square,
                                     accum_out=st1[:, DT + m:DT + m + 1])

        if DEBUG_TAPS and l == 0:
            nc.sync.dma_start(io["dbg_y1"].rearrange("(k p) t -> p k t", p=P),
                              xnxt[:])

        # y1 currently lives in xnxt; BN1 normalizes it in place into
        # xcur-for-ffn (xnxt holds y1; apply writes xb16 + xnxt fp32)
        sc1, sh1 = bn_reduce(f"a{l}", st1, g1_sb, be1_sb)
        for m in range(DT):
            nc.scalar.activation(xb16[:, m, :], xnxt[:, m, :], AF.Identity,
                                 bias=sh1[:, m:m + 1], scale=sc1[:, m:m + 1])
            nc.vector.tensor_scalar(out=xnxt[:, m, :], in0=xnxt[:, m, :],
                                    scalar1=sc1[:, m:m + 1],
                                    scalar2=sh1[:, m:m + 1],
                                    op0=ALU.mult, op1=ALU.add)
        xcur, xnxt = xnxt, xcur

        if DEBUG_TAPS and l == 0:
            nc.sync.dma_start(io["dbg_x2"].rearrange("(k p) t -> p k t", p=P),
                              xcur[:])

        w2_ch = [wchunk(w2_r[:, ts(h, 4), :], l, f"w2{l}_{h}") for h in range(8)]

        # ---- FFN1: h^T = relu(W1^T x^T + b1)
        for m in range(FMT):
            ps1 = ps.tile([P, TL], f32, tag="mm", name=f"ps1{l}_{m}")
            for k in range(DT):
                nc.tensor.matmul(ps1[:], w1_ch[m // 4][:, k, ts(m % 4, P)],
                                 xb16[:, k, :], start=(k == 0), stop=(k == DT - 1))
            nc.scalar.activation(ht[:, m, :], ps1[:], AF.Relu,
                                 bias=b1_sb[:, m:m + 1])

        if DEBUG_TAPS and l == 0:
            nc.sync.dma_start(io["dbg_h"].rearrange("(k p) t -> p k t", p=P),
                              ht[:])

        # ---- FFN2 + residual -> y2 with fused BN partial stats
        st2 = spool.tile([P, 16], f32, tag="st", name=f"st2_{l}")
        sqs2 = spool.tile([P, TL], f32, tag="sqs", name=f"sq2_{l}")
        for m in range(DT):
            ps2 = ps.tile([P, TL], f32, tag="mm", name=f"ps2{l}_{m}")
            for k in range(FMT):
                nc.tensor.matmul(ps2[:], w2_ch[k // 4][:, k % 4, ts(m, P)],
                                 ht[:, k, :], start=(k == 0), stop=(k == FMT - 1))
            nc.vector.scalar_tensor_tensor(
                out=xnxt[:, m, :], in0=ps2[:], scalar=1.0, in1=xcur[:, m, :],
                op0=ALU.mult, op1=ALU.add, accum_out=st2[:, m:m + 1])
            if USE_TTR:
                nc.vector.tensor_tensor_reduce(
                    out=sqs2[:], in0=xnxt[:, m, :], in1=xnxt[:, m, :], scale=1.0,
                    scalar=0.0, op0=ALU.mult, op1=ALU.add,
                    accum_out=st2[:, DT + m:DT + m + 1])
            else:
                nc.scalar.activation(sqs2[:], xnxt[:, m, :], AF.Square,
                                     accum_out=st2[:, DT + m:DT + m + 1])

        if DEBUG_TAPS and l == 0:
            nc.sync.dma_start(io["dbg_y2"].rearrange("(k p) t -> p k t", p=P),
                              xnxt[:])

        sc2, sh2 = bn_reduce(f"f{l}", st2, g2_sb, be2_sb)
        for m in range(DT):
            nc.scalar.activation(xb16[:, m, :], xnxt[:, m, :], AF.Identity,
                                 bias=sh2[:, m:m + 1], scale=sc2[:, m:m + 1])
            nc.vector.tensor_scalar(out=xnxt[:, m, :], in0=xnxt[:, m, :],
                                    scalar1=sc2[:, m:m + 1],
                                    scalar2=sh2[:, m:m + 1],
                                    op0=ALU.mult, op1=ALU.add)
        xcur, xnxt = xnxt, xcur

    # ---------------- output x^T local slice
    dmae.dma_start(io["out"].rearrange("(k p) t -> p k t", p=P), xcur[:])
    st_.close()


# ================================================================ host side

def make_in_maps(inputs):
    import ml_dtypes
    bf = lambda a: np.ascontiguousarray(np.asarray(a, dtype=np.float32)).astype(
        ml_dtypes.bfloat16)
    f = lambda a: np.ascontiguousarray(np.asarray(a), dtype=np.float32)
    seq = np.asarray(inputs["sequence"]).reshape(-1).astype(np.int16)
    emb = bf(inputs["emb"])
    pesT = np.ascontiguousarray(f(inputs["pes"]).T)            # [D, S]
    wq, wk, wv = bf(inputs["Wq"]), bf(inputs["Wk"]), bf(inputs["Wv"])
    wo, w1, w2 = bf(inputs["Wo"]), bf(inputs["W1"]), bf(inputs["W2"])
    bq, b1 = f(inputs["bq"]), f(inputs["b1"])
    g1, be1 = f(inputs["g1"]), f(inputs["be1"])
    g2, be2 = f(inputs["g2"]), f(inputs["be2"])

    in_maps = []
    for c in range(NC):
        loc = seq[c * TL:(c + 1) * TL]
        idx = np.ascontiguousarray(loc.reshape(TL // 16, 16).T)    # [16, TL/16]
        off = (c % 2) * TL
        in_maps.append({
            "emb": emb,
            "idx": idx,
            "pesT": np.ascontiguousarray(pesT[:, off:off + TL]),
            "wq": wq, "wk": wk, "wv": wv, "wo": wo, "w1": w1, "w2": w2,
            "bq": bq, "b1": b1,
            "g1": g1, "be1": be1, "g2": g2, "be2": be2,
        })
    return in_maps


_CACHE = {}


def _get_module():
    if "nc" not in _CACHE:
        _CACHE["nc"] = build_module()
    return _CACHE["nc"]


def kernel(**inputs):
    from concourse import bass_utils
    nc = _get_module()
    in_maps = make_in_maps(inputs)
    res = bass_utils.run_bass_kernel_spmd(nc, in_maps, list(range(NC)))
    full = np.concatenate(
        [np.asarray(res.results[c]["out"]) for c in range(NC)], axis=1)
    return np.ascontiguousarray(full.T).reshape(B, S, D).astype(np.float32)


# revision 36
# speedup vs baseline: 2.0978x; 1.1038x over previous
"""Trainium2 Bass kernel for a 6-layer post-BatchNorm transformer encoder.

Reference model:
  x = emb[seq] + pes                                  # [B,S,D] = [4,512,1024]
  6x: x = BN(x + attn(x)); x = BN(x + ffn(x))
  BN = per-channel batch stats over (B,S), eps=1e-3.

Sharding: token-sharded data parallel across 8 NeuronCores. Core c owns the
256 contiguous tokens [256c, 256c+256) = batch c//2, sequence half c%2. All
matmuls are local full-width (every core streams the full bf16 weights from
HBM in 1MB chunks through a ring); residual adds and BatchNorm application
are local. Cross-core communication per layer:
  - one pair AllGather (cores 2b,2b+1) of K^T and V (bf16, 1MB in / 2MB out)
    so attention sees the full 512-key sequence of its batch,
  - two 8KB AllGathers of per-core BN partial sums/sumsq (the only global
    coupling BatchNorm actually needs).
bk/bv/bo/b2 biases cancel mathematically (bk/bv through softmax rows summing
to 1, bo/b2 inside BN mean subtraction) and are dropped; bq and b1 are kept.

Numerics: all matmuls in bf16 (fp32 PSUM accumulation); x kept in fp32
master + bf16 matmul copy; BN statistics in fp32.

Layout: activations transposed [128 part, dtile, tokens]; weights natural
[Din, Dout] serve as lhsT. Attention per head pair packs the two heads at
partition bases 0/64 (row-group concurrency on the PE). Softmax sums ride
as a ones-column in the even head's V (PSUM rows 64) and a separate
ones-row matmul into PSUM row 96 for the odd head; the odd head's U lands
at PSUM rows 64:128 via output col-group 64, so every downstream copy is
partition-aligned. Embedding gather uses dma_gather(transpose=True), which
lands rows directly in the transposed layout.
"""

import os

import numpy as np

import concourse.bass as bass
import concourse.mybir as mybir
import concourse.tile as tile
from concourse import bacc
from concourse.bass import ts

# ---------------------------------------------------------------- dims
V, D, L, H, B, S = 32000, 1024, 6, 16, 4, 512
HD = D // H            # 64
DF = 4 * D             # 4096
EPS = 1e-3
NC = 8                 # cores
T = B * S              # 2048 tokens total
TL = T // NC           # 256 tokens per core
P = 128                # partitions
DT = D // P            # 8 d-tiles
FMT = DF // P          # 32 ffn1 m-tiles
SK = S // P            # 4 key chunks per batch

f32 = mybir.dt.float32
bf16 = mybir.dt.bfloat16
i16 = mybir.dt.int16
AF = mybir.ActivationFunctionType
ALU = mybir.AluOpType

ALLGRP = [list(range(NC))]
KVGRP = [[2 * b, 2 * b + 1] for b in range(B)]

N_LAYERS = int(os.environ.get("TRN_KERNEL_LAYERS", str(L)))
DEBUG_TAPS = os.environ.get("TRN_KERNEL_DEBUG", "0") == "1"
QSPLIT = os.environ.get("TRN_QSPLIT", "1") == "1"
FASTRECIP = os.environ.get("TRN_FASTRECIP", "1") == "1"
USE_TTR = os.environ.get("TRN_TTR", "0") == "1"

KVB = TL * D           # elements of the K^T block in the kv exchange
VWB = H * (HD + 1)     # 1040: per-token V row incl ones columns
VB2 = 2 * P * VWB      # elements of the interleaved V block
KVT = VB2 + KVB        # total elements per rank in the kv exchange


def build_module(n_layers=None):
    if n_layers is None:
        n_layers = N_LAYERS
    nc = bacc.Bacc("TRN2", target_bir_lowering=False, debug=False,
                   num_devices=NC)

    dt_ = nc.dram_tensor
    io = {
        "emb": dt_("emb", [V, D], bf16, kind="ExternalInput").ap(),
        "idx": dt_("idx", [16, TL // 16], i16, kind="ExternalInput").ap(),
        "pesT": dt_("pesT", [D, TL], f32, kind="ExternalInput").ap(),
        "wq": dt_("wq", [L, D, D], bf16, kind="ExternalInput").ap(),
        "wk": dt_("wk", [L, D, D], bf16, kind="ExternalInput").ap(),
        "wv": dt_("wv", [L, D, D], bf16, kind="ExternalInput").ap(),
        "wo": dt_("wo", [L, D, D], bf16, kind="ExternalInput").ap(),
        "w1": dt_("w1", [L, D, DF], bf16, kind="ExternalInput").ap(),
        "w2": dt_("w2", [L, DF, D], bf16, kind="ExternalInput").ap(),
        "bq": dt_("bq", [L, P, DT], f32, kind="ExternalInput").ap(),
        "b1": dt_("b1", [L, P, FMT], f32, kind="ExternalInput").ap(),
        "g1": dt_("g1", [L, P, DT], f32, kind="ExternalInput").ap(),
        "be1": dt_("be1", [L, P, DT], f32, kind="ExternalInput").ap(),
        "g2": dt_("g2", [L, P, DT], f32, kind="ExternalInput").ap(),
        "be2": dt_("be2", [L, P, DT], f32, kind="ExternalInput").ap(),
        "out": dt_("out", [D, TL], f32, kind="ExternalOutput").ap(),
    }
    if DEBUG_TAPS:
        for nm, shp in [("dbg_x", [D, TL]), ("dbg_y1", [D, TL]),
                        ("dbg_x2", [D, TL]), ("dbg_y2", [D, TL])]:
            io[nm] = dt_(nm, shp, f32, kind="ExternalOutput").ap()
        for nm, shp in [("dbg_q", [D, TL]), ("dbg_k", [D, S]),
                        ("dbg_vsb", [P, SK * H * (HD + 1)]),
                        ("dbg_attnT", [D, TL]), ("dbg_h", [DF, TL])]:
            io[nm] = dt_(nm, shp, bf16, kind="ExternalOutput").ap()

    with tile.TileContext(nc) as tc:
        _build(tc, n_layers, io)
    nc.compile()
    return nc


def _build(tc, n_layers, io):
    from contextlib import ExitStack
    nc = tc.nc
    att_scale = 1.0 / np.sqrt(HD)
    dmae = nc.scalar if QSPLIT else nc.sync

    st_ = ExitStack()
    persist = st_.enter_context(tc.tile_pool(name="persist", bufs=1))
    wpool = st_.enter_context(tc.tile_pool(name="wpool", bufs=8))
    ppool = st_.enter_context(tc.tile_pool(name="ppool", bufs=2))
    epool = st_.enter_context(tc.tile_pool(name="epool", bufs=10))
    spool = st_.enter_context(tc.tile_pool(name="spool", bufs=2))
    ps = st_.enter_context(tc.tile_pool(name="ps", bufs=3, space="PSUM"))
    drin = st_.enter_context(tc.tile_pool(name="drin", bufs=2, space="DRAM"))
    drout = st_.enter_context(tc.tile_pool(name="drout", bufs=2, space="DRAM"))

    # ---------------- persistent tiles
    xf32a = persist.tile([P, DT, TL], f32, name="xf32a")
    xf32b = persist.tile([P, DT, TL], f32, name="xf32b")
    xb16 = persist.tile([P, DT, TL], bf16, name="xb16")
    qT = persist.tile([P, DT, TL], bf16, name="qT")
    kloc = persist.tile([P, DT, TL], bf16, name="kloc")
    vloc = persist.tile([P, 2, H, HD + 1], bf16, name="vloc")
    kT = persist.tile([P, DT, S], bf16, name="kT")
    vsb = persist.tile([P, SK, H, HD + 1], bf16, name="vsb")
    attnT = persist.tile([P, DT, TL], bf16, name="attnT")
    ht = persist.tile([P, FMT, TL], bf16, name="ht")
    onesb = persist.tile([P, P], bf16, name="onesb")
    idxs = persist.tile([P, TL // 16], i16, name="idxs")

    nc.vector.memset(onesb[:], 1.0)
    nc.vector.memset(vloc[:, :, :, HD:HD + 1], 1.0)
    for r_ in range(P // 16):
        nc.sync.dma_start(idxs[16 * r_:16 * (r_ + 1), :], io["idx"])

    # ---------------- embedding: x^T = (emb[seq])^T + pes^T
    pes_sb = spool.tile([P, DT, TL], f32, tag="pes", bufs=1, name="pes_sb")
    dmae.dma_start(pes_sb[:], io["pesT"].rearrange("(k p) t -> p k t", p=P))
    gt = spool.tile([P, DT, TL], bf16, tag="gt", bufs=1, name="gt")
    nc.gpsimd.dma_gather(
        out_ap=gt[:], in_ap=io["emb"], idxs_ap=idxs[:],
        num_idxs=TL, num_idxs_reg=TL, elem_size=D, transpose=True)
    for k in range(DT):
        nc.vector.tensor_tensor(out=xf32a[:, k, :], in0=gt[:, k, :],
                                in1=pes_sb[:, k, :], op=ALU.add)
    nc.vector.tensor_copy(xb16[:], xf32a[:])

    if DEBUG_TAPS:
        nc.sync.dma_start(io["dbg_x"].rearrange("(k p) t -> p k t", p=P),
                          xf32a[:])

    xcur = xf32a
    xnxt = xf32b

    # ---------------- per-layer param loads (small)
    def load_params(l):
        bq_sb = ppool.tile([P, DT], f32, tag="bq", name=f"bq{l}")
        b1_sb = ppool.tile([P, FMT], f32, tag="b1", name=f"b1{l}")
        g1_sb = ppool.tile([P, DT], f32, tag="g1", name=f"g1{l}")
        be1_sb = ppool.tile([P, DT], f32, tag="be1", name=f"be1{l}")
        g2_sb = ppool.tile([P, DT], f32, tag="g2", name=f"g2{l}")
        be2_sb = ppool.tile([P, DT], f32, tag="be2", name=f"be2{l}")
        dmae.dma_start(bq_sb[:], io["bq"][l])
        dmae.dma_start(b1_sb[:], io["b1"][l])
        dmae.dma_start(g1_sb[:], io["g1"][l])
        dmae.dma_start(be1_sb[:], io["be1"][l])
        dmae.dma_start(g2_sb[:], io["g2"][l])
        dmae.dma_start(be2_sb[:], io["be2"][l])
        return bq_sb, b1_sb, g1_sb, be1_sb, g2_sb, be2_sb

    # weight chunk loader: returns [P, DT, 512] (half the out-cols of a DxD
    # weight) or [P, 4, D] (4 k-tiles of w2)
    def wchunk(src_ap, l, nm):
        t = wpool.tile(list(src_ap.shape), bf16, tag="w", name=nm)
        nc.sync.dma_start(t[:], src_ap)
        return t

    # BN stats -> AllGather -> sc/sh
    def bn_reduce(lbl, stats, g_sb, be_sb):
        sti = drin.tile([P * 16], f32, tag="sti", name=f"sti{lbl}")
        sto = drout.tile([NC * P * 16], f32, tag="sto", addr_space="Shared",
                         name=f"sto{lbl}")
        dmae.dma_start(sti[:].rearrange("(p s) -> p s", p=P), stats[:])
        nc.gpsimd.collective_compute(
            "AllGather", ALU.bypass, replica_groups=ALLGRP,
            ins=[sti[:].opt()], outs=[sto[:].opt()])
        ld = spool.tile([P, NC, 16], f32, tag="ld", name=f"ld{lbl}")
        dmae.dma_start(ld[:], sto[:].rearrange("(r p s) -> p r s", p=P, s=16))
        u1 = spool.tile([P, 4, 16], f32, tag="u1", name=f"u1{lbl}")
        nc.vector.tensor_tensor(out=u1[:], in0=ld[:, 0:4, :], in1=ld[:, 4:8, :],
                                op=ALU.add)
        u2 = spool.tile([P, 2, 16], f32, tag="u2", name=f"u2{lbl}")
        nc.vector.tensor_tensor(out=u2[:], in0=u1[:, 0:2, :], in1=u1[:, 2:4, :],
                                op=ALU.add)
        tot = spool.tile([P, 16], f32, tag="tot", name=f"tot{lbl}")
        nc.vector.tensor_tensor(out=tot[:], in0=u2[:, 0, :], in1=u2[:, 1, :],
                                op=ALU.add)
        mean = spool.tile([P, DT], f32, tag="mean", name=f"mean{lbl}")
        nc.vector.tensor_scalar_mul(mean[:], tot[:, 0:DT], 1.0 / T)
        msq = spool.tile([P, DT], f32, tag="msq", name=f"msq{lbl}")
        nc.vector.tensor_tensor(out=msq[:], in0=mean[:], in1=mean[:], op=ALU.mult)
        veps = spool.tile([P, DT], f32, tag="veps", name=f"veps{lbl}")
        nc.vector.scalar_tensor_tensor(out=veps[:], in0=tot[:, DT:16],
                                       scalar=1.0 / T, in1=msq[:],
                                       op0=ALU.mult, op1=ALU.subtract)
        nc.vector.tensor_scalar_add(veps[:], veps[:], EPS)
        rec = spool.tile([P, DT], f32, tag="rec", name=f"rec{lbl}")
        nc.vector.reciprocal(rec[:], veps[:])
        rstd = spool.tile([P, DT], f32, tag="rstd", name=f"rstd{lbl}")
        nc.scalar.sqrt(rstd[:], rec[:])
        sc = spool.tile([P, DT], f32, tag="sc", name=f"sc{lbl}")
        nc.vector.tensor_tensor(out=sc[:], in0=g_sb[:], in1=rstd[:], op=ALU.mult)
        sh = spool.tile([P, DT], f32, tag="sh", name=f"sh{lbl}")
        nc.vector.tensor_tensor(out=sh[:], in0=mean[:], in1=sc[:], op=ALU.mult)
        nc.vector.tensor_tensor(out=sh[:], in0=be_sb[:], in1=sh[:], op=ALU.subtract)
        return sc, sh

    # ---------------- layers
    for l in range(n_layers):
        bq_sb, b1_sb, g1_sb, be1_sb, g2_sb, be2_sb = load_params(l)

        wk_r = io["wk"][l].rearrange("(k p) m -> p k m", p=P)
        wv_r = io["wv"][l].rearrange("(k p) m -> p k m", p=P)
        wq_r = io["wq"][l].rearrange("(k p) m -> p k m", p=P)
        wo_r = io["wo"][l].rearrange("(k p) m -> p k m", p=P)
        w1_r = io["w1"][l].rearrange("(k p) m -> p k m", p=P)
        w2_r = io["w2"][l].rearrange("(k p) m -> p k m", p=P)

        wk_ch = [wchunk(wk_r[:, :, ts(h, 512)], l, f"wk{l}_{h}") for h in range(2)]
        wv_ch = [wchunk(wv_r[:, :, ts(h, 512)], l, f"wv{l}_{h}") for h in range(2)]
        wq_ch = [wchunk(wq_r[:, :, ts(h, 512)], l, f"wq{l}_{h}") for h in range(2)]

        # ---- K projection (local tokens): K^T = Wk^T x^T
        for g in range(DT):
            psk = ps.tile([P, TL], f32, tag="mm", name=f"psk{l}_{g}")
            for k in range(DT):
                nc.tensor.matmul(psk[:], wk_ch[g // 4][:, k, ts(g % 4, P)],
                                 xb16[:, k, :], start=(k == 0), stop=(k == DT - 1))
            nc.vector.tensor_copy(kloc[:, g, :], psk[:])

        # ---- V projection, token-major: V = x W_v (x tiles stationary)
        for mt in range(2):
            for nb in range(4):
                psv = ps.tile([P, TL], f32, tag="mm", name=f"psv{l}_{mt}_{nb}")
                for k in range(DT):
                    nc.tensor.matmul(
                        psv[:], xb16[:, k, ts(mt, P)],
                        wv_ch[nb // 2][:, k, ts(nb % 2, 256)],
                        start=(k == 0), stop=(k == DT - 1))
                nc.vector.tensor_copy(
                    vloc[:, mt, 4 * nb:4 * nb + 4, 0:HD],
                    psv[:].rearrange("p (h x) -> p h x", h=4))

        # ---- ship local K/V, gather the batch pair's full K/V
        kvi = drin.tile([KVT], bf16, tag="kvi", name=f"kvi{l}")
        kvo = drout.tile([2 * KVT], bf16, tag="kvo", name=f"kvo{l}")
        dmae.dma_start(
            kvi[0:VB2].rearrange("(p q) -> p q", p=P),
            vloc[:].rearrange("p a h x -> p (a h x)"))
        dmae.dma_start(
            kvi[VB2:KVT].rearrange("(g p t) -> p g t", g=DT, p=P), kloc[:])
        nc.gpsimd.collective_compute(
            "AllGather", ALU.bypass, replica_groups=KVGRP,
            ins=[kvi[:].opt()], outs=[kvo[:].opt()])

        # ---- Q projection (overlaps the AllGather)
        for g in range(DT):
            psq = ps.tile([P, TL], f32, tag="mm", name=f"psq{l}_{g}")
            for k in range(DT):
                nc.tensor.matmul(psq[:], wq_ch[g // 4][:, k, ts(g % 4, P)],
                                 xb16[:, k, :], start=(k == 0), stop=(k == DT - 1))
            nc.vector.tensor_scalar_add(qT[:, g, :], psq[:], bq_sb[:, g:g + 1])

        # ---- land gathered K/V (all byte-contiguous: V ships pre-interleaved)
        for hf in range(2):
            base = hf * KVT
            dmae.dma_start(
                kT[:, :, ts(hf, TL)],
                kvo[base + VB2:base + KVT].rearrange(
                    "(g p t) -> p g t", g=DT, p=P))
            dmae.dma_start(
                vsb[:, 2 * hf:2 * hf + 2, :, :].rearrange(
                    "p a h x -> p (a h x)"),
                kvo[base:base + VB2].rearrange("(p q) -> p q", p=P))

        if DEBUG_TAPS and l == 0:
            nc.sync.dma_start(io["dbg_q"].rearrange("(k p) t -> p k t", p=P), qT[:])
            nc.sync.dma_start(io["dbg_k"].rearrange("(k p) t -> p k t", p=P), kT[:])
            nc.sync.dma_start(
                io["dbg_vsb"].rearrange("p (a b c) -> p a b c", a=SK, b=H), vsb[:])

        wo_ch = [wchunk(wo_r[:, :, ts(h, 512)], l, f"wo{l}_{h}") for h in range(2)]

        # ---- attention. Phase A: all pairs' scores + exp (exp batched over
        # 2 key-chunks); PE streams scores back-to-back while scalar exps.
        eall = []
        for g in range(DT):
            epair = [[None, None], [None, None]]
            for kcb in range(2):
                sst = [None, None]
                for tw in range(2):
                    sst[tw] = ps.tile([P, 2, TL], f32, tag="s", bufs=2,
                                      name=f"pss{l}_{g}_{kcb}_{tw}")
                for j in range(2):
                    for tw in range(2):
                        hp = 64 * tw
                        nc.tensor.matmul(
                            sst[tw][:, j, :],
                            kT[hp:hp + HD, g, ts(2 * kcb + j, P)],
                            qT[hp:hp + HD, g, :], start=True, stop=True)
                for tw in range(2):
                    et = epool.tile([P, 2, TL], bf16, tag="e", bufs=18,
                                    name=f"et{l}_{g}_{kcb}_{tw}")
                    nc.scalar.activation(et[:], sst[tw][:], AF.Exp,
                                         scale=att_scale)
                    epair[tw][kcb] = et
            eall.append(epair)

        # Phase B1: per pair, U accumulation; unnormalized U and the sumexp
        # rows drain to SBUF so the reciprocals batch across all 8 pairs.
        serow = spool.tile([P, DT, TL], f32, tag="serow", name=f"serow{l}")
        rsall = spool.tile([P, DT, TL], bf16, tag="rsall", name=f"rsall{l}")
        usbs = []
        for g in range(DT):
            epair = eall[g]
            bankA = ps.tile([P, TL], f32, tag="u", name=f"bA{l}_{g}")
            bankB = ps.tile([P, TL], f32, tag="u", name=f"bB{l}_{g}")
            for kc in range(SK):
                fl, ll = (kc == 0), (kc == SK - 1)
                ee = epair[0][kc // 2][:, kc % 2, :]
                eo = epair[1][kc // 2][:, kc % 2, :]
                nc.tensor.matmul(bankA[0:HD + 1, :],
                                 vsb[:, kc, 2 * g, 0:HD + 1], ee,
                                 start=fl, stop=ll)
                nc.tensor.matmul(bankB[64:128, :],
                                 vsb[:, kc, 2 * g + 1, 0:HD], eo,
                                 start=fl, stop=ll)
                nc.tensor.matmul(bankA[96:97, :], onesb[:, 0:1], eo,
                                 start=fl, stop=ll, tile_position=(0, 96))
            usbE = epool.tile([P, TL], bf16, tag="usb", bufs=18,
                              name=f"uE{l}_{g}")
            usbO = epool.tile([P, TL], bf16, tag="usb", bufs=18,
                              name=f"uO{l}_{g}")
            nc.vector.tensor_copy(usbE[0:64, :], bankA[0:64, :])
            nc.vector.tensor_copy(usbO[64:128, :], bankB[64:128, :])
            nc.vector.tensor_copy(serow[HD:HD + 1, g, :], bankA[HD:HD + 1, :])
            nc.vector.tensor_copy(serow[96:97, g, :], bankA[96:97, :])
            usbs.append((usbE, usbO))
        with nc.allow_low_precision(reason="softmax 1/sumexp as bf16"):
            nc.vector.reciprocal(rsall[HD:HD + 1, :, :], serow[HD:HD + 1, :, :])
            nc.vector.reciprocal(rsall[96:97, :, :], serow[96:97, :, :])

        # Phase B2: broadcast 1/sumexp to the head's 64 partitions (PE) and
        # normalize into attnT.
        for g in range(DT):
            usbE, usbO = usbs[g]
            psr = ps.tile([P, TL], f32, tag="u", name=f"psr{l}_{g}")
            nc.tensor.matmul(psr[0:64, :], onesb[HD:HD + 1, 0:64],
                             rsall[HD:HD + 1, g, :], start=True, stop=True)
            nc.tensor.matmul(psr[64:128, :], onesb[96:97, 0:64],
                             rsall[96:97, g, :], start=True, stop=True,
                             tile_position=(96, 64))
            nc.vector.tensor_tensor(out=attnT[0:64, g, :], in0=usbE[0:64, :],
                                    in1=psr[0:64, :], op=ALU.mult)
            nc.vector.tensor_tensor(out=attnT[64:128, g, :], in0=usbO[64:128, :],
                                    in1=psr[64:128, :], op=ALU.mult)

        if DEBUG_TAPS and l == 0:
            nc.sync.dma_start(
                io["dbg_attnT"].rearrange("(k p) t -> p k t", p=P), attnT[:])

        w1_ch = [wchunk(w1_r[:, :, ts(h, 512)], l, f"w1{l}_{h}") for h in range(8)]

        # ---- Wo + residual -> y1 (fp32) with fused BN partial stats
        st1 = spool.tile([P, 16], f32, tag="st", name=f"st1_{l}")
        sqs = spool.tile([P, TL], f32, tag="sqs", name=f"sq1_{l}")
        for m in range(DT):
            pso = ps.tile([P, TL], f32, tag="mm", name=f"pso{l}_{m}")
            for k in range(DT):
                nc.tensor.matmul(pso[:], wo_ch[m // 4][:, k, ts(m % 4, P)],
                                 attnT[:, k, :], start=(k == 0), stop=(k == DT - 1))
            nc.vector.scalar_tensor_tensor(
                out=xnxt[:, m, :], in0=pso[:], scalar=1.0, in1=xcur[:, m, :],
                op0=ALU.mult, op1=ALU.add, accum_out=st1[:, m:m + 1])
            if USE_TTR:
                nc.vector.tensor_tensor_reduce(
                    out=sqs[:], in0=xnxt[:, m, :], in1=xnxt[:, m, :], scale=1.0,
                    scalar=0.0, op0=ALU.mult, op1=ALU.add,
                    accum_out=st1[:, DT + m:DT + m + 1])
            else:
                nc.scalar.activation(sqs[:], xnxt[:, m, :], AF.Square,
                                     accum_out=st1[:, DT + m:DT + m + 1])

        if DEBUG_TAPS and l == 0:
            nc.sync.dma_start(io["dbg_y1"].rearrange("(k p) t -> p k t", p=P),
                              xnxt[:])

        # y1 currently lives in xnxt; BN1 normalizes it in place into
        # xcur-for-ffn (xnxt holds y1; apply writes xb16 + xnxt fp32)
        sc1, sh1 = bn_reduce(f"a{l}", st1, g1_sb, be1_sb)
        for m in range(DT):
            nc.scalar.activation(xb16[:, m, :], xnxt[:, m, :], AF.Identity,
                                 bias=sh1[:, m:m + 1], scale=sc1[:, m:m + 1])
            nc.vector.tensor_scalar(out=xnxt[:, m, :], in0=xnxt[:, m, :],
                                    scalar1=sc1[:, m:m + 1],
                                    scalar2=sh1[:, m:m + 1],
                                    op0=ALU.mult, op1=ALU.add)
        xcur, xnxt = xnxt, xcur

        if DEBUG_TAPS and l == 0:
            nc.sync.dma_start(io["dbg_x2"].rearrange("(k p) t -> p k t", p=P),
                              xcur[:])

        w2_ch = [wchunk(w2_r[:, ts(h, 4), :], l, f"w2{l}_{h}") for h in range(8)]

        # ---- FFN1: h^T = relu(W1^T x^T + b1)
        for m in range(FMT):
            ps1 = ps.tile([P, TL], f32, tag="mm", name=f"ps1{l}_{m}")
            for k in range(DT):
                nc.tensor.matmul(ps1[:], w1_ch[m // 4][:, k, ts(m % 4, P)],
                                 xb16[:, k, :], start=(k == 0), stop=(k == DT - 1))
            nc.scalar.activation(ht[:, m, :], ps1[:], AF.Relu,
                                 bias=b1_sb[:, m:m + 1])

        if DEBUG_TAPS and l == 0:
            nc.sync.dma_start(io["dbg_h"].rearrange("(k p) t -> p k t", p=P),
                              ht[:])

        # ---- FFN2 + residual -> y2 with fused BN partial stats
        st2 = spool.tile([P, 16], f32, tag="st", name=f"st2_{l}")
        sqs2 = spool.tile([P, TL], f32, tag="sqs", name=f"sq2_{l}")
        for m in range(DT):
            ps2 = ps.tile([P, TL], f32, tag="mm", name=f"ps2{l}_{m}")
            for k in range(FMT):
                nc.tensor.matmul(ps2[:], w2_ch[k // 4][:, k % 4, ts(m, P)],
                                 ht[:, k, :], start=(k == 0), stop=(k == FMT - 1))
            nc.vector.scalar_tensor_tensor(
                out=xnxt[:, m, :], in0=ps2[:], scalar=1.0, in1=xcur[:, m, :],
                op0=ALU.mult, op1=ALU.add, accum_out=st2[:, m:m + 1])
            if USE_TTR:
                nc.vector.tensor_tensor_reduce(
                    out=sqs2[:], in0=xnxt[:, m, :], in1=xnxt[:, m, :], scale=1.0,
                    scalar=0.0, op0=ALU.mult, op1=ALU.add,
                    accum_out=st2[:, DT + m:DT + m + 1])
            else:
                nc.scalar.activation(sqs2[:], xnxt[:, m, :], AF.Square,
                                     accum_out=st2[:, DT + m:DT + m + 1])

        if DEBUG_TAPS and l == 0:
            nc.sync.dma_start(io["dbg_y2"].rearrange("(k p) t -> p k t", p=P),
                              xnxt[:])

        sc2, sh2 = bn_reduce(f"f{l}", st2, g2_sb, be2_sb)
        for m in range(DT):
            nc.scalar.activation(xb16[:, m, :], xnxt[:, m, :], AF.Identity,
                                 bias=sh2[:, m:m + 1], scale=sc2[:, m:m + 1])
            nc.vector.tensor_scalar(out=xnxt[:, m, :], in0=xnxt[:, m, :],
                                    scalar1=sc2[:, m:m + 1],
                                    scalar2=sh2[:, m:m + 1],
                                    op0=ALU.mult, op1=ALU.add)
        xcur, xnxt = xnxt, xcur

    # ---------------- output x^T local slice
    dmae.dma_start(io["out"].rearrange("(k p) t -> p k t", p=P), xcur[:])
    st_.close()


# ================================================================ host side

def make_in_maps(inputs):
    import ml_dtypes
    bf = lambda a: np.ascontiguousarray(np.asarray(a, dtype=np.float32)).astype(
        ml_dtypes.bfloat16)
    f = lambda a: np.ascontiguousarray(np.asarray(a), dtype=np.float32)
    seq = np.asarray(inputs["sequence"]).reshape(-1).astype(np.int16)
    emb = bf(inputs["emb"])
    pesT = np.ascontiguousarray(f(inputs["pes"]).T)            # [D, S]
    wq, wk, wv = bf(inputs["Wq"]), bf(inputs["Wk"]), bf(inputs["Wv"])
    wo, w1, w2 = bf(inputs["Wo"]), bf(inputs["W1"]), bf(inputs["W2"])
    pt = lambda a, m: np.ascontiguousarray(
        f(a).reshape(L, m, P).transpose(0, 2, 1))   # [L, P, m] with ch = m*128+p
    bq, b1 = pt(inputs["bq"], DT), pt(inputs["b1"], FMT)
    g1, be1 = pt(inputs["g1"], DT), pt(inputs["be1"], DT)
    g2, be2 = pt(inputs["g2"], DT), pt(inputs["be2"], DT)

    in_maps = []
    for c in range(NC):
        loc = seq[c * TL:(c + 1) * TL]
        idx = np.ascontiguousarray(loc.reshape(TL // 16, 16).T)    # [16, TL/16]
        off = (c % 2) * TL
        in_maps.append({
            "emb": emb,
            "idx": idx,
            "pesT": np.ascontiguousarray(pesT[:, off:off + TL]),
            "wq": wq, "wk": wk, "wv": wv, "wo": wo, "w1": w1, "w2": w2,
            "bq": bq, "b1": b1,
            "g1": g1, "be1": be1, "g2": g2, "be2": be2,
        })
    return in_maps


_CACHE = {}


def _get_module():
    if "nc" not in _CACHE:
        _CACHE["nc"] = build_module()
    return _CACHE["nc"]


def kernel(**inputs):
    from concourse import bass_utils
    nc = _get_module()
    in_maps = make_in_maps(inputs)
    res = bass_utils.run_bass_kernel_spmd(nc, in_maps, list(range(NC)))
    full = np.concatenate(
        [np.asarray(res.results[c]["out"]) for c in range(NC)], axis=1)
    return np.ascontiguousarray(full.T).reshape(B, S, D).astype(np.float32)


# revision 40
# speedup vs baseline: 2.3817x; 1.1353x over previous
"""Trainium2 Bass kernel for a 6-layer post-BatchNorm transformer encoder.

Reference model:
  x = emb[seq] + pes                                  # [B,S,D] = [4,512,1024]
  6x: x = BN(x + attn(x)); x = BN(x + ffn(x))
  BN = per-channel batch stats over (B,S), eps=1e-3.

Sharding: token-sharded data parallel across 8 NeuronCores. Core c owns the
256 contiguous tokens [256c, 256c+256) = batch c//2, sequence half c%2. All
matmuls are local full-width (every core streams the full bf16 weights from
HBM in 1MB chunks through a ring); residual adds and BatchNorm application
are local. Cross-core communication per layer:
  - one pair AllGather (cores 2b,2b+1) of K^T and V (bf16, 1MB in / 2MB out)
    so attention sees the full 512-key sequence of its batch,
  - two 8KB AllGathers of per-core BN partial sums/sumsq (the only global
    coupling BatchNorm actually needs).
bk/bv/bo/b2 biases cancel mathematically (bk/bv through softmax rows summing
to 1, bo/b2 inside BN mean subtraction) and are dropped; bq and b1 are kept.

Numerics: all matmuls in bf16 (fp32 PSUM accumulation); x kept in fp32
master + bf16 matmul copy; BN statistics in fp32.

Layout: activations transposed [128 part, dtile, tokens]; weights natural
[Din, Dout] serve as lhsT. Attention per head pair packs the two heads at
partition bases 0/64 (row-group concurrency on the PE). Softmax sums ride
as a ones-column in the even head's V (PSUM rows 64) and a separate
ones-row matmul into PSUM row 96 for the odd head; the odd head's U lands
at PSUM rows 64:128 via output col-group 64, so every downstream copy is
partition-aligned. Embedding gather uses dma_gather(transpose=True), which
lands rows directly in the transposed layout.
"""

import os

import numpy as np

import concourse.bass as bass
import concourse.mybir as mybir
import concourse.tile as tile
from concourse import bacc
from concourse.bass import ts

# ---------------------------------------------------------------- dims
V, D, L, H, B, S = 32000, 1024, 6, 16, 4, 512
HD = D // H            # 64
DF = 4 * D             # 4096
EPS = 1e-3
NC = 8                 # cores
T = B * S              # 2048 tokens total
TL = T // NC           # 256 tokens per core
P = 128                # partitions
DT = D // P            # 8 d-tiles
FMT = DF // P          # 32 ffn1 m-tiles
SK = S // P            # 4 key chunks per batch

f32 = mybir.dt.float32
bf16 = mybir.dt.bfloat16
i16 = mybir.dt.int16
AF = mybir.ActivationFunctionType
ALU = mybir.AluOpType

ALLGRP = [list(range(NC))]
KVGRP = [[2 * b, 2 * b + 1] for b in range(B)]

N_LAYERS = int(os.environ.get("TRN_KERNEL_LAYERS", str(L)))
DEBUG_TAPS = os.environ.get("TRN_KERNEL_DEBUG", "0") == "1"
QSPLIT = os.environ.get("TRN_QSPLIT", "1") == "1"
FASTRECIP = os.environ.get("TRN_FASTRECIP", "1") == "1"
USE_TTR = os.environ.get("TRN_TTR", "0") == "1"

KVB = TL * D           # elements of the K^T block in the kv exchange
VWB = H * (HD + 1)     # 1040: per-token V row incl ones columns
VB2 = 2 * P * VWB      # elements of the interleaved V block
KVT = VB2 + KVB        # total elements per rank in the kv exchange


def build_module(n_layers=None):
    if n_layers is None:
        n_layers = N_LAYERS
    nc = bacc.Bacc("TRN2", target_bir_lowering=False, debug=False,
                   num_devices=NC)

    dt_ = nc.dram_tensor
    io = {
        "emb": dt_("emb", [V, D], bf16, kind="ExternalInput").ap(),
        "idx": dt_("idx", [16, TL // 16], i16, kind="ExternalInput").ap(),
        "pesT": dt_("pesT", [D, TL], f32, kind="ExternalInput").ap(),
        "wq": dt_("wq", [L, D, D], bf16, kind="ExternalInput").ap(),
        "wk": dt_("wk", [L, D, D], bf16, kind="ExternalInput").ap(),
        "wv": dt_("wv", [L, D, D], bf16, kind="ExternalInput").ap(),
        "wo": dt_("wo", [L, D, D], bf16, kind="ExternalInput").ap(),
        "w1": dt_("w1", [L, D, DF], bf16, kind="ExternalInput").ap(),
        "w2": dt_("w2", [L, DF, D], bf16, kind="ExternalInput").ap(),
        "bq": dt_("bq", [L, P, DT], f32, kind="ExternalInput").ap(),
        "b1": dt_("b1", [L, P, FMT], f32, kind="ExternalInput").ap(),
        "g1": dt_("g1", [L, P, DT], f32, kind="ExternalInput").ap(),
        "be1": dt_("be1", [L, P, DT], f32, kind="ExternalInput").ap(),
        "g2": dt_("g2", [L, P, DT], f32, kind="ExternalInput").ap(),
        "be2": dt_("be2", [L, P, DT], f32, kind="ExternalInput").ap(),
        "out": dt_("out", [D, TL], f32, kind="ExternalOutput").ap(),
    }
    if DEBUG_TAPS:
        for nm, shp in [("dbg_x", [D, TL]), ("dbg_y1", [D, TL]),
                        ("dbg_x2", [D, TL]), ("dbg_y2", [D, TL])]:
            io[nm] = dt_(nm, shp, f32, kind="ExternalOutput").ap()
        for nm, shp in [("dbg_q", [D, TL]), ("dbg_k", [D, S]),
                        ("dbg_vsb", [P, SK * H * (HD + 1)]),
                        ("dbg_attnT", [D, TL]), ("dbg_h", [DF, TL])]:
            io[nm] = dt_(nm, shp, bf16, kind="ExternalOutput").ap()

    with tile.TileContext(nc) as tc:
        _build(tc, n_layers, io)
    nc.compile()
    return nc


def _build(tc, n_layers, io):
    from contextlib import ExitStack
    nc = tc.nc
    att_scale = 1.0 / np.sqrt(HD)
    dmae = nc.scalar if QSPLIT else nc.sync

    st_ = ExitStack()
    persist = st_.enter_context(tc.tile_pool(name="persist", bufs=1))
    wpool = st_.enter_context(tc.tile_pool(name="wpool", bufs=10))
    ppool = st_.enter_context(tc.tile_pool(name="ppool", bufs=2))
    epool = st_.enter_context(tc.tile_pool(name="epool", bufs=10))
    spool = st_.enter_context(tc.tile_pool(name="spool", bufs=2))
    ps = st_.enter_context(tc.tile_pool(name="ps", bufs=2, space="PSUM"))
    drin = st_.enter_context(tc.tile_pool(name="drin", bufs=2, space="DRAM"))
    drout = st_.enter_context(tc.tile_pool(name="drout", bufs=2, space="DRAM"))

    # ---------------- persistent tiles
    xf32a = persist.tile([P, DT, TL], f32, name="xf32a")
    xf32b = persist.tile([P, DT, TL], f32, name="xf32b")
    xb16 = persist.tile([P, DT, TL], bf16, name="xb16")
    qT = persist.tile([P, DT, TL], bf16, name="qT")
    kloc = persist.tile([P, DT, TL], bf16, name="kloc")
    vloc = persist.tile([P, 2, H, HD + 1], bf16, name="vloc")
    kT = persist.tile([P, DT, S], bf16, name="kT")
    vsb = persist.tile([P, SK, H, HD + 1], bf16, name="vsb")
    attnT = persist.tile([P, DT, TL], bf16, name="attnT")
    ht = persist.tile([P, FMT, TL], bf16, name="ht")
    onesb = persist.tile([P, P], bf16, name="onesb")
    idxs = persist.tile([P, TL // 16], i16, name="idxs")

    nc.vector.memset(onesb[:], 1.0)
    nc.vector.memset(vloc[:, :, :, HD:HD + 1], 1.0)
    for r_ in range(P // 16):
        nc.sync.dma_start(idxs[16 * r_:16 * (r_ + 1), :], io["idx"])

    # ---------------- embedding: x^T = (emb[seq])^T + pes^T
    pes_sb = spool.tile([P, DT, TL], f32, tag="pes", bufs=1, name="pes_sb")
    dmae.dma_start(pes_sb[:], io["pesT"].rearrange("(k p) t -> p k t", p=P))
    gt = spool.tile([P, DT, TL], bf16, tag="gt", bufs=1, name="gt")
    nc.gpsimd.dma_gather(
        out_ap=gt[:], in_ap=io["emb"], idxs_ap=idxs[:],
        num_idxs=TL, num_idxs_reg=TL, elem_size=D, transpose=True)
    for k in range(DT):
        nc.vector.tensor_tensor(out=xf32a[:, k, :], in0=gt[:, k, :],
                                in1=pes_sb[:, k, :], op=ALU.add)
    nc.vector.tensor_copy(xb16[:], xf32a[:])

    if DEBUG_TAPS:
        nc.sync.dma_start(io["dbg_x"].rearrange("(k p) t -> p k t", p=P),
                          xf32a[:])

    xcur = xf32a
    xnxt = xf32b

    # ---------------- per-layer param loads (small)
    def load_params(l):
        bq_sb = ppool.tile([P, DT], f32, tag="bq", name=f"bq{l}")
        b1_sb = ppool.tile([P, FMT], f32, tag="b1", name=f"b1{l}")
        g1_sb = ppool.tile([P, DT], f32, tag="g1", name=f"g1{l}")
        be1_sb = ppool.tile([P, DT], f32, tag="be1", name=f"be1{l}")
        g2_sb = ppool.tile([P, DT], f32, tag="g2", name=f"g2{l}")
        be2_sb = ppool.tile([P, DT], f32, tag="be2", name=f"be2{l}")
        dmae.dma_start(bq_sb[:], io["bq"][l])
        dmae.dma_start(b1_sb[:], io["b1"][l])
        dmae.dma_start(g1_sb[:], io["g1"][l])
        dmae.dma_start(be1_sb[:], io["be1"][l])
        dmae.dma_start(g2_sb[:], io["g2"][l])
        dmae.dma_start(be2_sb[:], io["be2"][l])
        return bq_sb, b1_sb, g1_sb, be1_sb, g2_sb, be2_sb

    # weight chunk loader: returns [P, DT, 512] (half the out-cols of a DxD
    # weight) or [P, 4, D] (4 k-tiles of w2)
    def wchunk(src_ap, l, nm):
        t = wpool.tile(list(src_ap.shape), bf16, tag="w", name=nm)
        nc.sync.dma_start(t[:], src_ap)
        return t

    # BN stats -> AllGather -> sc/sh
    def bn_reduce(lbl, stats, g_sb, be_sb):
        sti = drin.tile([P * 16], f32, tag="sti", name=f"sti{lbl}")
        sto = drout.tile([NC * P * 16], f32, tag="sto", addr_space="Shared",
                         name=f"sto{lbl}")
        nc.gpsimd.dma_start(sti[:].rearrange("(p s) -> p s", p=P), stats[:])
        nc.gpsimd.collective_compute(
            "AllGather", ALU.bypass, replica_groups=ALLGRP,
            ins=[sti[:].opt()], outs=[sto[:].opt()])
        ld = spool.tile([P, NC, 16], f32, tag="ld", name=f"ld{lbl}")
        nc.gpsimd.dma_start(ld[:], sto[:].rearrange("(r p s) -> p r s", p=P, s=16))
        u1 = spool.tile([P, 4, 16], f32, tag="u1", name=f"u1{lbl}")
        nc.vector.tensor_tensor(out=u1[:], in0=ld[:, 0:4, :], in1=ld[:, 4:8, :],
                                op=ALU.add)
        u2 = spool.tile([P, 2, 16], f32, tag="u2", name=f"u2{lbl}")
        nc.vector.tensor_tensor(out=u2[:], in0=u1[:, 0:2, :], in1=u1[:, 2:4, :],
                                op=ALU.add)
        tot = spool.tile([P, 16], f32, tag="tot", name=f"tot{lbl}")
        nc.vector.tensor_tensor(out=tot[:], in0=u2[:, 0, :], in1=u2[:, 1, :],
                                op=ALU.add)
        mean = spool.tile([P, DT], f32, tag="mean", name=f"mean{lbl}")
        nc.vector.tensor_scalar_mul(mean[:], tot[:, 0:DT], 1.0 / T)
        msq = spool.tile([P, DT], f32, tag="msq", name=f"msq{lbl}")
        nc.vector.tensor_tensor(out=msq[:], in0=mean[:], in1=mean[:], op=ALU.mult)
        veps = spool.tile([P, DT], f32, tag="veps", name=f"veps{lbl}")
        nc.vector.scalar_tensor_tensor(out=veps[:], in0=tot[:, DT:16],
                                       scalar=1.0 / T, in1=msq[:],
                                       op0=ALU.mult, op1=ALU.subtract)
        nc.vector.tensor_scalar_add(veps[:], veps[:], EPS)
        rec = spool.tile([P, DT], f32, tag="rec", name=f"rec{lbl}")
        nc.vector.reciprocal(rec[:], veps[:])
        rstd = spool.tile([P, DT], f32, tag="rstd", name=f"rstd{lbl}")
        nc.scalar.sqrt(rstd[:], rec[:])
        sc = spool.tile([P, DT], f32, tag="sc", name=f"sc{lbl}")
        nc.vector.tensor_tensor(out=sc[:], in0=g_sb[:], in1=rstd[:], op=ALU.mult)
        sh = spool.tile([P, DT], f32, tag="sh", name=f"sh{lbl}")
        nc.vector.tensor_tensor(out=sh[:], in0=mean[:], in1=sc[:], op=ALU.mult)
        nc.vector.tensor_tensor(out=sh[:], in0=be_sb[:], in1=sh[:], op=ALU.subtract)
        return sc, sh

    # ---------------- layers
    for l in range(n_layers):
        bq_sb, b1_sb, g1_sb, be1_sb, g2_sb, be2_sb = load_params(l)

        wk_r = io["wk"][l].rearrange("(k p) m -> p k m", p=P)
        wv_r = io["wv"][l].rearrange("(k p) m -> p k m", p=P)
        wq_r = io["wq"][l].rearrange("(k p) m -> p k m", p=P)
        wo_r = io["wo"][l].rearrange("(k p) m -> p k m", p=P)
        w1_r = io["w1"][l].rearrange("(k p) m -> p k m", p=P)
        w2_r = io["w2"][l].rearrange("(k p) m -> p k m", p=P)

        wk_ch = [wchunk(wk_r[:, :, ts(h, 512)], l, f"wk{l}_{h}") for h in range(2)]
        wv_ch = [wchunk(wv_r[:, :, ts(h, 512)], l, f"wv{l}_{h}") for h in range(2)]
        wq_ch = [wchunk(wq_r[:, :, ts(h, 512)], l, f"wq{l}_{h}") for h in range(2)]

        # ---- K projection (local tokens): K^T = Wk^T x^T
        for g in range(DT):
            psk = ps.tile([P, TL], f32, tag="mm", name=f"psk{l}_{g}")
            for k in range(DT):
                nc.tensor.matmul(psk[:], wk_ch[g // 4][:, k, ts(g % 4, P)],
                                 xb16[:, k, :], start=(k == 0), stop=(k == DT - 1))
            nc.vector.tensor_copy(kloc[:, g, :], psk[:])

        # ---- V projection, token-major: V = x W_v (x tiles stationary)
        for mt in range(2):
            for nb in range(4):
                psv = ps.tile([P, TL], f32, tag="mm", name=f"psv{l}_{mt}_{nb}")
                for k in range(DT):
                    nc.tensor.matmul(
                        psv[:], xb16[:, k, ts(mt, P)],
                        wv_ch[nb // 2][:, k, ts(nb % 2, 256)],
                        start=(k == 0), stop=(k == DT - 1))
                nc.vector.tensor_copy(
                    vloc[:, mt, 4 * nb:4 * nb + 4, 0:HD],
                    psv[:].rearrange("p (h x) -> p h x", h=4))

        # ---- ship local K/V, gather the batch pair's full K/V
        kvi = drin.tile([KVT], bf16, tag="kvi", name=f"kvi{l}")
        kvo = drout.tile([2 * KVT], bf16, tag="kvo", name=f"kvo{l}")
        dmae.dma_start(
            kvi[0:VB2].rearrange("(p q) -> p q", p=P),
            vloc[:].rearrange("p a h x -> p (a h x)"))
        dmae.dma_start(
            kvi[VB2:KVT].rearrange("(g p t) -> p g t", g=DT, p=P), kloc[:])
        nc.gpsimd.collective_compute(
            "AllGather", ALU.bypass, replica_groups=KVGRP,
            ins=[kvi[:].opt()], outs=[kvo[:].opt()])

        # ---- Q projection (overlaps the AllGather)
        for g in range(DT):
            psq = ps.tile([P, TL], f32, tag="mm", name=f"psq{l}_{g}")
            for k in range(DT):
                nc.tensor.matmul(psq[:], wq_ch[g // 4][:, k, ts(g % 4, P)],
                                 xb16[:, k, :], start=(k == 0), stop=(k == DT - 1))
            nc.vector.tensor_scalar_add(qT[:, g, :], psq[:], bq_sb[:, g:g + 1])

        # ---- land gathered K/V (all byte-contiguous: V ships pre-interleaved)
        for hf in range(2):
            base = hf * KVT
            dmae.dma_start(
                kT[:, :, ts(hf, TL)],
                kvo[base + VB2:base + KVT].rearrange(
                    "(g p t) -> p g t", g=DT, p=P))
            dmae.dma_start(
                vsb[:, 2 * hf:2 * hf + 2, :, :].rearrange(
                    "p a h x -> p (a h x)"),
                kvo[base:base + VB2].rearrange("(p q) -> p q", p=P))

        if DEBUG_TAPS and l == 0:
            nc.sync.dma_start(io["dbg_q"].rearrange("(k p) t -> p k t", p=P), qT[:])
            nc.sync.dma_start(io["dbg_k"].rearrange("(k p) t -> p k t", p=P), kT[:])
            nc.sync.dma_start(
                io["dbg_vsb"].rearrange("p (a b c) -> p a b c", a=SK, b=H), vsb[:])

        wo_ch = [wchunk(wo_r[:, :, ts(h, 512)], l, f"wo{l}_{h}") for h in range(2)]

        # ---- attention. Phase A: all pairs' scores + exp (exp batched over
        # 2 key-chunks); PE streams scores back-to-back while scalar exps.
        eall = []
        for g in range(DT):
            epair = [[None, None], [None, None]]
            for kcb in range(2):
                sst = [None, None]
                for tw in range(2):
                    sst[tw] = ps.tile([P, 2, TL], f32, tag="s", bufs=2,
                                      name=f"pss{l}_{g}_{kcb}_{tw}")
                for j in range(2):
                    for tw in range(2):
                        hp = 64 * tw
                        nc.tensor.matmul(
                            sst[tw][:, j, :],
                            kT[hp:hp + HD, g, ts(2 * kcb + j, P)],
                            qT[hp:hp + HD, g, :], start=True, stop=True)
                for tw in range(2):
                    et = epool.tile([P, 2, TL], bf16, tag="e", bufs=18,
                                    name=f"et{l}_{g}_{kcb}_{tw}")
                    nc.scalar.activation(et[:], sst[tw][:], AF.Exp,
                                         scale=att_scale)
                    epair[tw][kcb] = et
            eall.append(epair)

        # Phase B1: per pair, U accumulation (M=64 each head); all 16 softmax
        # sums land on distinct PSUM partition strips of two shared banks so
        # ONE reciprocal per bank covers 8 of them in parallel lanes.
        se0 = ps.tile([P, 2, TL], f32, tag="se", name=f"se0_{l}")
        se1 = ps.tile([P, 2, TL], f32, tag="se", name=f"se1_{l}")
        usbs = []
        for g in range(DT):
            epair = eall[g]
            seb = se0 if g < 4 else se1
            strip = 32 * (g % 4)
            bankA = ps.tile([P, TL], f32, tag="u", name=f"bA{l}_{g}")
            bankB = ps.tile([P, TL], f32, tag="u", name=f"bB{l}_{g}")
            for kc in range(SK):
                fl, ll = (kc == 0), (kc == SK - 1)
                ee = epair[0][kc // 2][:, kc % 2, :]
                eo = epair[1][kc // 2][:, kc % 2, :]
                nc.tensor.matmul(bankA[0:HD, :],
                                 vsb[:, kc, 2 * g, 0:HD], ee,
                                 start=fl, stop=ll)
                nc.tensor.matmul(bankB[64:128, :],
                                 vsb[:, kc, 2 * g + 1, 0:HD], eo,
                                 start=fl, stop=ll)
                nc.tensor.matmul(seb[strip:strip + 1, 0, :], onesb[:, 0:1],
                                 ee, start=fl, stop=ll, tile_position=(0, strip))
                nc.tensor.matmul(seb[strip:strip + 1, 1, :], onesb[:, 0:1],
                                 eo, start=fl, stop=ll, tile_position=(0, strip))
            usbE = epool.tile([P, TL], bf16, tag="usb", bufs=18,
                              name=f"uE{l}_{g}")
            usbO = epool.tile([P, TL], bf16, tag="usb", bufs=18,
                              name=f"uO{l}_{g}")
            nc.vector.tensor_copy(usbE[0:64, :], bankA[0:64, :])
            nc.vector.tensor_copy(usbO[64:128, :], bankB[64:128, :])
            usbs.append((usbE, usbO))
        rs0 = spool.tile([P, 2, TL], bf16, tag="rs16", name=f"rs0_{l}")
        rs1 = spool.tile([P, 2, TL], bf16, tag="rs16", name=f"rs1_{l}")
        with nc.allow_low_precision(reason="softmax 1/sumexp as bf16"):
            nc.vector.reciprocal(rs0[:], se0[:])
            nc.vector.reciprocal(rs1[:], se1[:])

        # Phase B2: broadcast 1/sumexp to the head's 64 partitions (PE) and
        # normalize into attnT.
        for g in range(DT):
            usbE, usbO = usbs[g]
            rsx = rs0 if g < 4 else rs1
            strip = 32 * (g % 4)
            psr = ps.tile([P, TL], f32, tag="se", name=f"psr{l}_{g}")
            nc.tensor.matmul(psr[0:64, :], onesb[strip:strip + 1, 0:64],
                             rsx[strip:strip + 1, 0, :], start=True, stop=True,
                             tile_position=(strip, 0))
            nc.tensor.matmul(psr[64:128, :], onesb[strip:strip + 1, 0:64],
                             rsx[strip:strip + 1, 1, :], start=True, stop=True,
                             tile_position=(strip, 64))
            nc.vector.tensor_tensor(out=attnT[0:64, g, :], in0=usbE[0:64, :],
                                    in1=psr[0:64, :], op=ALU.mult)
            nc.vector.tensor_tensor(out=attnT[64:128, g, :], in0=usbO[64:128, :],
                                    in1=psr[64:128, :], op=ALU.mult)

        if DEBUG_TAPS and l == 0:
            nc.sync.dma_start(
                io["dbg_attnT"].rearrange("(k p) t -> p k t", p=P), attnT[:])

        w1_ch = [wchunk(w1_r[:, :, ts(h, 512)], l, f"w1{l}_{h}") for h in range(8)]

        # ---- Wo + residual -> y1 (fp32) with fused BN partial stats
        st1 = spool.tile([P, 16], f32, tag="st", name=f"st1_{l}")
        sqs = spool.tile([P, TL], f32, tag="sqs", name=f"sq1_{l}")
        for m in range(DT):
            pso = ps.tile([P, TL], f32, tag="mm", name=f"pso{l}_{m}")
            for k in range(DT):
                nc.tensor.matmul(pso[:], wo_ch[m // 4][:, k, ts(m % 4, P)],
                                 attnT[:, k, :], start=(k == 0), stop=(k == DT - 1))
            nc.vector.scalar_tensor_tensor(
                out=xnxt[:, m, :], in0=pso[:], scalar=1.0, in1=xcur[:, m, :],
                op0=ALU.mult, op1=ALU.add, accum_out=st1[:, m:m + 1])
            if USE_TTR:
                nc.vector.tensor_tensor_reduce(
                    out=sqs[:], in0=xnxt[:, m, :], in1=xnxt[:, m, :], scale=1.0,
                    scalar=0.0, op0=ALU.mult, op1=ALU.add,
                    accum_out=st1[:, DT + m:DT + m + 1])
            else:
                nc.scalar.activation(sqs[:], xnxt[:, m, :], AF.Square,
                                     accum_out=st1[:, DT + m:DT + m + 1])

        if DEBUG_TAPS and l == 0:
            nc.sync.dma_start(io["dbg_y1"].rearrange("(k p) t -> p k t", p=P),
                              xnxt[:])

        # y1 currently lives in xnxt; BN1 normalizes it in place into
        # xcur-for-ffn (xnxt holds y1; apply writes xb16 + xnxt fp32)
        sc1, sh1 = bn_reduce(f"a{l}", st1, g1_sb, be1_sb)
        for m in range(DT):
            nc.scalar.activation(xb16[:, m, :], xnxt[:, m, :], AF.Identity,
                                 bias=sh1[:, m:m + 1], scale=sc1[:, m:m + 1])
            nc.vector.tensor_scalar(out=xnxt[:, m, :], in0=xnxt[:, m, :],
                                    scalar1=sc1[:, m:m + 1],
                                    scalar2=sh1[:, m:m + 1],
                                    op0=ALU.mult, op1=ALU.add)
        xcur, xnxt = xnxt, xcur

        if DEBUG_TAPS and l == 0:
            nc.sync.dma_start(io["dbg_x2"].rearrange("(k p) t -> p k t", p=P),
                              xcur[:])

        w2_ch = [wchunk(w2_r[:, ts(h, 4), :], l, f"w2{l}_{h}") for h in range(8)]

        # ---- FFN1: h^T = relu(W1^T x^T + b1)
        for m in range(FMT):
            ps1 = ps.tile([P, TL], f32, tag="mm", name=f"ps1{l}_{m}")
            for k in range(DT):
                nc.tensor.matmul(ps1[:], w1_ch[m // 4][:, k, ts(m % 4, P)],
                                 xb16[:, k, :], start=(k == 0), stop=(k == DT - 1))
            nc.scalar.activation(ht[:, m, :], ps1[:], AF.Relu,
                                 bias=b1_sb[:, m:m + 1])

        if DEBUG_TAPS and l == 0:
            nc.sync.dma_start(io["dbg_h"].rearrange("(k p) t -> p k t", p=P),
                              ht[:])

        # ---- FFN2 + residual -> y2 with fused BN partial stats
        st2 = spool.tile([P, 16], f32, tag="st", name=f"st2_{l}")
        sqs2 = spool.tile([P, TL], f32, tag="sqs", name=f"sq2_{l}")
        for m in range(DT):
            ps2 = ps.tile([P, TL], f32, tag="mm", name=f"ps2{l}_{m}")
            for k in range(FMT):
                nc.tensor.matmul(ps2[:], w2_ch[k // 4][:, k % 4, ts(m, P)],
                                 ht[:, k, :], start=(k == 0), stop=(k == FMT - 1))
            nc.vector.scalar_tensor_tensor(
                out=xnxt[:, m, :], in0=ps2[:], scalar=1.0, in1=xcur[:, m, :],
                op0=ALU.mult, op1=ALU.add, accum_out=st2[:, m:m + 1])
            if USE_TTR:
                nc.vector.tensor_tensor_reduce(
                    out=sqs2[:], in0=xnxt[:, m, :], in1=xnxt[:, m, :], scale=1.0,
                    scalar=0.0, op0=ALU.mult, op1=ALU.add,
                    accum_out=st2[:, DT + m:DT + m + 1])
            else:
                nc.scalar.activation(sqs2[:], xnxt[:, m, :], AF.Square,
                                     accum_out=st2[:, DT + m:DT + m + 1])

        if DEBUG_TAPS and l == 0:
            nc.sync.dma_start(io["dbg_y2"].rearrange("(k p) t -> p k t", p=P),
                              xnxt[:])

        sc2, sh2 = bn_reduce(f"f{l}", st2, g2_sb, be2_sb)
        for m in range(DT):
            nc.scalar.activation(xb16[:, m, :], xnxt[:, m, :], AF.Identity,
                                 bias=sh2[:, m:m + 1], scale=sc2[:, m:m + 1])
            nc.vector.tensor_scalar(out=xnxt[:, m, :], in0=xnxt[:, m, :],
                                    scalar1=sc2[:, m:m + 1],
                                    scalar2=sh2[:, m:m + 1],
                                    op0=ALU.mult, op1=ALU.add)
        xcur, xnxt = xnxt, xcur

    # ---------------- output x^T local slice
    dmae.dma_start(io["out"].rearrange("(k p) t -> p k t", p=P), xcur[:])
    st_.close()


# ================================================================ host side

def make_in_maps(inputs):
    import ml_dtypes
    bf = lambda a: np.ascontiguousarray(np.asarray(a, dtype=np.float32)).astype(
        ml_dtypes.bfloat16)
    f = lambda a: np.ascontiguousarray(np.asarray(a), dtype=np.float32)
    seq = np.asarray(inputs["sequence"]).reshape(-1).astype(np.int16)
    emb = bf(inputs["emb"])
    pesT = np.ascontiguousarray(f(inputs["pes"]).T)            # [D, S]
    wq, wk, wv = bf(inputs["Wq"]), bf(inputs["Wk"]), bf(inputs["Wv"])
    wo, w1, w2 = bf(inputs["Wo"]), bf(inputs["W1"]), bf(inputs["W2"])
    pt = lambda a, m: np.ascontiguousarray(
        f(a).reshape(L, m, P).transpose(0, 2, 1))   # [L, P, m] with ch = m*128+p
    bq, b1 = pt(inputs["bq"], DT), pt(inputs["b1"], FMT)
    g1, be1 = pt(inputs["g1"], DT), pt(inputs["be1"], DT)
    g2, be2 = pt(inputs["g2"], DT), pt(inputs["be2"], DT)

    in_maps = []
    for c in range(NC):
        loc = seq[c * TL:(c + 1) * TL]
        idx = np.ascontiguousarray(loc.reshape(TL // 16, 16).T)    # [16, TL/16]
        off = (c % 2) * TL
        in_maps.append({
            "emb": emb,
            "idx": idx,
            "pesT": np.ascontiguousarray(pesT[:, off:off + TL]),
            "wq": wq, "wk": wk, "wv": wv, "wo": wo, "w1": w1, "w2": w2,
            "bq": bq, "b1": b1,
            "g1": g1, "be1": be1, "g2": g2, "be2": be2,
        })
    return in_maps


_CACHE = {}


def _get_module():
    if "nc" not in _CACHE:
        _CACHE["nc"] = build_module()
    return _CACHE["nc"]


def kernel(**inputs):
    from concourse import bass_utils
    nc = _get_module()
    in_maps = make_in_maps(inputs)
    res = bass_utils.run_bass_kernel_spmd(nc, in_maps, list(range(NC)))
    full = np.concatenate(
        [np.asarray(res.results[c]["out"]) for c in range(NC)], axis=1)
    return np.ascontiguousarray(full.T).reshape(B, S, D).astype(np.float32)
